# revision 1
# baseline (speedup 1.0000x reference)
"""GAT layer kernel for Trainium2 (8 NeuronCores, SPMD, no collectives).

Math (reference):
    att = h @ h.T / sqrt(256)
    A = softmax(where(adj>0, att, -9e15), axis=1)
    A = (A + I) * 0.5; rows < k (k = nnz(adj[:,0])) overwritten with I
    out = relu(A @ (h @ W.T + b))

Algorithm here (flash-style, attention matrix never materialized/scaled):
  - rows [0,k): out = relu(h@W.T + b)  (identity rows)
  - rows [k,N): out = relu(0.5*num/S + 0.5*h@W.T|row + b), where
        num = sum_j mask[i,j]*exp(att[i,j]) * (h@W.T)[j],
        S   = sum_j mask[i,j]*exp(att[i,j])
    Masking by multiply after exp (exact zeros); no row-max subtraction
    needed: att in [-7, 22] for this input family, exp stays in f32 range.
  - Transposed layout: each core computes att_T[j, i] for its own output
    rows i (sharded on host), j contracted over all 8192 via PSUM
    accumulation; numerator and denominator come from one matmul chain
    against [h_new | 1].

Sharding: identity rows and attention rows each split evenly across the 8
cores; every core runs the same NEFF on different input slices.
"""

import math
import os
import sys

for _p in ("/opt/trn_rl_repo", "/root/.axon_site/_ro/trn_rl_repo"):
    if os.path.isdir(_p) and _p not in sys.path:
        sys.path.append(_p)

import numpy as np
import orjson

import concourse.bass as bass
import concourse.tile as tile
from concourse import mybir

F32 = mybir.dt.float32
F16 = mybir.dt.float16
BF16 = mybir.dt.bfloat16
I8 = mybir.dt.int8

N = 8192
D = 256
NCORES = 8
NJC = N // 128  # 64 j-chunks
SCALE = 1.0 / 16.0


def _spill_waits(nc, max_sync=2):
    """Walrus rejects instructions with more sync commands than the lowered
    ISA struct can hold (2 for compute/DMA, 1 for NoOp/Drain). Tile can emit
    more. Move excess waits onto injected NoOps preceding the instruction
    (same engine, executes in order, so semantics are preserved)."""
    bir = orjson.loads(nc.to_json_bytes())
    for fn in bir["functions"]:
        for bb in fn["blocks"]:
            insts = bb.get("instructions") or []
            out = []
            for inst in insts:
                si = inst.get("sync_info")
                if si:
                    waits = si.get("on_wait") or []
                    upds = si.get("on_update") or []
                    lim = 1 if inst["opcode"] in ("NoOp", "Drain") else max_sync
                    cap = max(0, lim - len(upds))
                    if len(waits) > cap:
                        extra = waits[cap:]
                        si["on_wait"] = waits[:cap]
                        for ci, w in enumerate(extra):
                            out.append(
                                {
                                    "engine": inst["engine"],
                                    "ins": [],
                                    "outs": [],
                                    "name": f"{inst['name']}-sw{ci}",
                                    "opcode": "NoOp",
                                    "sync_info": {"on_wait": [w], "on_update": []},
                                    "debug": inst.get("debug", 0),
                                }
                            )
                out.append(inst)
            bb["instructions"] = out
    blob = orjson.dumps(bir)
    nc.to_json_bytes = lambda: blob


def _build(nid, nis, repeat=1, hnew_mode="compute", abl=(), depth=2, att_bufs=2, merge_ps=False, work_bufs=4):
    """Build the SPMD program. nid/nis = number of 128-row identity /
    attention sub-tiles per core. OWN = (nid+nis)*128 own rows per core.
    repeat: unroll the whole kernel body N times (benchmark use)."""
    nown = nid + nis
    own = nown * 128
    rpad = nis * 128

    nc = bass.Bass("TRN2", target_bir_lowering=False, debug=False, num_devices=NCORES)

    hT_d = nc.dram_tensor("hT", [D, N], F16, kind="ExternalInput").ap()
    hTo_d = nc.dram_tensor("hTo", [D, own], F16, kind="ExternalInput").ap()
    WT_d = nc.dram_tensor("WT", [D, 256], F16, kind="ExternalInput").ap()
    bb_d = nc.dram_tensor("bb", [128, 256], F32, kind="ExternalInput").ap()
    if nis:
        mT_d = nc.dram_tensor("mT", [N, rpad], I8, kind="ExternalInput").ap()
    if hnew_mode == "dram":
        hn_d = nc.dram_tensor("hn", [N, 257], BF16, kind="ExternalInput").ap()
    out_d = nc.dram_tensor("out", [own, 256], F32, kind="ExternalOutput").ap()

    with tile.TileContext(nc) as tc:
        pp = None  # set below
        with (
            tc.tile_pool(name="big", bufs=1) as big,
            tc.tile_pool(name="hnp", bufs=1) as hnp,
            tc.tile_pool(name="gout", bufs=1) as gout,
            tc.tile_pool(name="work", bufs=work_bufs) as work,
            tc.tile_pool(name="fin", bufs=2) as fin,
            tc.tile_pool(name="ps", bufs=2, space="PSUM") as pp0,
            tc.tile_pool(name="att_ps", bufs=att_bufs, space="PSUM") as app,
            tc.tile_pool(name="acc", bufs=1, space="PSUM") as accp,
        ):
            pp = app if merge_ps else pp0
            for _rep in range(repeat):
              # --- persistent loads ---
              # hT as 2 d-chunks x 4 column-chunks of 2048 (fewer DMAs --
              # HWDGE per-DMA overhead is ~0.5us)
              hTt = [[None] * 4 for _ in range(2)]
              for dchunk in range(2):
                  for cc in range(4):
                      t = big.tile([128, 2048], F16, tag=f"hT{dchunk}_{cc}")
                      nc.sync.dma_start(
                          t[:],
                          hT_d[
                              dchunk * 128 : (dchunk + 1) * 128,
                              cc * 2048 : (cc + 1) * 2048,
                          ],
                      )
                      hTt[dchunk][cc] = t
              hTo_t = []
              WT_t = []
              for dchunk in range(2):
                  t = big.tile([128, own], F16, tag=f"hTo{dchunk}")
                  nc.sync.dma_start(t[:], hTo_d[dchunk * 128 : (dchunk + 1) * 128, :])
                  hTo_t.append(t)
                  t = big.tile([128, 256], F16, tag=f"WT{dchunk}")
                  nc.sync.dma_start(t[:], WT_d[dchunk * 128 : (dchunk + 1) * 128, :])
                  WT_t.append(t)
              bb_t = big.tile([128, 256], F32, tag="bb")
              nc.sync.dma_start(bb_t[:], bb_d[:, :])

              def hT_slice(dchunk, jc):
                  return hTt[dchunk][jc // 16][:, (jc % 16) * 128 : (jc % 16 + 1) * 128]

              # --- own phase: h_new for own rows ---
              # identity tiles -> out rows directly; attention tiles -> g
              g_t = []
              if "no_own" in abl:
                  for t_i in range(nid, nown):
                      g = gout.tile([128, 256], F32, tag=f"g{t_i - nid}")
                      nc.vector.memset(g[:], 0.5)
                      g_t.append(g)
              for t_i in range(0 if "no_own" in abl else nown):
                  ps = pp.tile([128, 256], F32, tag="att_ps" if merge_ps else "hn_ps")
                  for dchunk in range(2):
                      nc.tensor.matmul(
                          ps[:],
                          hTo_t[dchunk][:, t_i * 128 : (t_i + 1) * 128],
                          WT_t[dchunk][:],
                          start=(dchunk == 0),
                          stop=(dchunk == 1),
                      )
                  if t_i < nid:
                      tmp = fin.tile([128, 256], F32, tag="idtmp")
                      nc.vector.tensor_tensor(
                          tmp[:], ps[:], bb_t[:], op=mybir.AluOpType.add
                      )
                      o_t = fin.tile([128, 256], F32, tag="ido")
                      nc.vector.tensor_scalar_max(o_t[:], tmp[:], 0.0)
                      nc.sync.dma_start(
                          out_d[t_i * 128 : (t_i + 1) * 128, :], o_t[:]
                      )
                  else:
                      g = gout.tile([128, 256], F32, tag=f"g{t_i - nid}")
                      nc.vector.scalar_tensor_tensor(
                          g[:],
                          ps[:],
                          0.5,
                          bb_t[:],
                          op0=mybir.AluOpType.mult,
                          op1=mybir.AluOpType.add,
                      )
                      g_t.append(g)

              if nis:
                  # --- h_new phase: h_new_plus[jc] = [h@W.T | 1] bf16 ---
                  hnew = []
                  if hnew_mode == "dram":
                      hnb = hnp.tile([128, NJC * 257], BF16, tag="hnewbig")
                      hn_r = hn_d.rearrange("(a p) w -> p a w", p=128)
                      for c2 in range(2):
                          nc.sync.dma_start(
                              hnb[:, c2 * 32 * 257 : (c2 + 1) * 32 * 257].rearrange(
                                  "p (a w) -> p a w", a=32
                              ),
                              hn_r[:, c2 * 32 : (c2 + 1) * 32, :],
                          )
                      hnew = [hnb[:, jc * 257 : (jc + 1) * 257] for jc in range(NJC)]
                  for jc in range(NJC if hnew_mode != "dram" else 0):
                      hp = hnp.tile([128, 257], BF16, tag=f"hnew{jc}")
                      if False:
                          pass
                      else:
                          ps = pp.tile([128, 256], F32, tag="att_ps" if merge_ps else "hn_ps")
                          for dchunk in range(2):
                              nc.tensor.matmul(
                                  ps[:],
                                  hT_slice(dchunk, jc),
                                  WT_t[dchunk][:],
                                  start=(dchunk == 0),
                                  stop=(dchunk == 1),
                              )
                          if jc % 2 == 0:
                              nc.vector.tensor_copy(hp[:, 0:256], ps[:])
                          else:
                              nc.scalar.copy(hp[:, 0:256], ps[:])
                          nc.vector.memset(hp[:, 256:257], 1.0)
                      hnew.append(hp)

                  # --- mask preload: [128, 64*rpad] i8, 4 big DMAs ---
                  if "no_att" in abl:
                      pass
                  elif "no_mask_dma" not in abl:
                      mbig = big.tile([128, NJC * rpad], I8, tag="mbig")
                      mT_r = mT_d.rearrange("(a p) w -> p a w", p=128)
                      for c4 in range(4):
                          nc.sync.dma_start(
                              mbig[:, c4 * 16 * rpad : (c4 + 1) * 16 * rpad].rearrange(
                                  "p (a w) -> p a w", a=16
                              ),
                              mT_r[:, c4 * 16 : (c4 + 1) * 16, :],
                          )

                  # --- attention phase ---
                  for ig in range(0 if "no_att" in abl else math.ceil(nis / 4)):
                      s0 = ig * 4
                      s1 = min(s0 + 4, nis)
                      iw = (s1 - s0) * 128  # width of this i-group
                      i_lo = s0 * 128
                      s_active = [s0] if "one_second" in abl else list(range(s0, s1))
                      acc = {}
                      for s in s_active:
                          acc_t = accp.tile([128, 257], F32, tag=f"acc{s - s0}")
                          acc[s - s0] = acc_t
                      # software pipeline: 2nd matmul for jc emitted DEPTH
                      # iterations later so PE doesn't wait on exp->mask chain
                      DEPTH = depth
                      pend = []

                      def emit_second(jc, em_t):
                          for s in s_active:
                              nc.tensor.matmul(
                                  acc[s - s0][:],
                                  em_t[:, (s - s0) * 128 : (s - s0 + 1) * 128],
                                  hnew[jc][:],
                                  start=(jc == 0),
                                  stop=(jc == NJC - 1),
                              )

                      for jc in range(NJC):
                          aps = app.tile([128, 512], F32, tag="att_ps")
                          ndch = 1 if "one_dchunk" in abl else 2
                          for dchunk in range(ndch):
                              nc.tensor.matmul(
                                  aps[:, 0:iw],
                                  hT_slice(dchunk, jc),
                                  hTo_t[dchunk][
                                      :, (nid * 128 + i_lo) : (nid * 128 + i_lo + iw)
                                  ],
                                  start=(dchunk == 0),
                                  stop=(dchunk == ndch - 1),
                              )
                          e_t = work.tile([128, 512], BF16, tag="e")
                          nc.scalar.activation(
                              e_t[:, 0:iw],
                              aps[:, 0:iw],
                              mybir.ActivationFunctionType.Copy
                              if "no_exp" in abl
                              else mybir.ActivationFunctionType.Exp,
                              scale=SCALE,
                          )
                          if "no_mask_dma" in abl:
                              if jc == 0:
                                  mfix = big.tile([128, 512], I8, tag="mfix")
                                  nc.vector.memset(mfix[:, 0:iw], 1)
                              m_sl = mfix[:, 0:iw]
                          else:
                              m_sl = mbig[:, jc * rpad + i_lo : jc * rpad + i_lo + iw]
                          if "no_mask_tt" in abl:
                              em_t = e_t
                          else:
                              em_t = work.tile([128, 512], BF16, tag="em")
                              nc.vector.tensor_tensor(
                                  em_t[:, 0:iw], e_t[:, 0:iw], m_sl,
                                  op=mybir.AluOpType.mult,
                              )
                          pend.append((jc, em_t))
                          if len(pend) > DEPTH:
                              emit_second(*pend.pop(0))
                      for item in pend:
                          emit_second(*item)
                      for s in s_active:
                          a = acc[s - s0]
                          recip = fin.tile([128, 1], F32, tag="recip")
                          nc.vector.reciprocal(recip[:], a[:, 256:257])
                          hr = fin.tile([128, 1], F32, tag="hr")
                          nc.vector.tensor_scalar_mul(hr[:], recip[:], 0.5)
                          tmp = fin.tile([128, 256], F32, tag="atmp")
                          nc.vector.scalar_tensor_tensor(
                              tmp[:],
                              a[:, 0:256],
                              hr[:],
                              g_t[s][:],
                              op0=mybir.AluOpType.mult,
                              op1=mybir.AluOpType.add,
                          )
                          o_t = fin.tile([128, 256], F32, tag="ao")
                          nc.vector.tensor_scalar_max(o_t[:], tmp[:], 0.0)
                          nc.sync.dma_start(
                              out_d[(nid + s) * 128 : (nid + s + 1) * 128, :], o_t[:]
                          )

    _spill_waits(nc)
    return nc


_CACHE = {}


def _prepare(h, adj, W, b):
    """Host-side sharding. Returns (nc, in_maps, assemble) where assemble
    takes the list of per-core 'out' arrays and produces the full output."""
    h = np.asarray(h, dtype=np.float32)
    adj = np.asarray(adj)
    W = np.asarray(W, dtype=np.float32)
    b = np.asarray(b, dtype=np.float32)

    k = int(np.count_nonzero(adj[:, 0]))
    nid = (k + NCORES * 128 - 1) // (NCORES * 128)  # id 128-tiles per core
    nis = (N - k + NCORES * 128 - 1) // (NCORES * 128)  # att 128-tiles per core
    key = (nid, nis)
    if key not in _CACHE:
        _CACHE[key] = _build(nid, nis)
    nc = _CACHE[key]

    kid = nid * 128  # padded id rows per core
    rpad = nis * 128  # padded att rows per core
    own = kid + rpad

    hT16 = np.ascontiguousarray(h.T).astype(np.float16)  # [D, N]
    WT16 = np.ascontiguousarray(W.T).astype(np.float16)
    bb = np.broadcast_to(b, (128, 256)).astype(np.float32).copy()
    adj8 = (adj != 0).view(np.int8) if adj.dtype == np.bool_ else (adj != 0)
    adj8 = adj8.view(np.int8) if adj8.dtype == np.bool_ else adj8.astype(np.int8)

    in_maps = []
    row_lists = []
    for c in range(NCORES):
        id_rows = np.arange(c * kid, (c + 1) * kid)
        id_valid = id_rows < k
        id_rows = np.where(id_valid, id_rows, 0)
        att_rows = np.arange(k + c * rpad, k + (c + 1) * rpad)
        att_valid = att_rows < N
        att_rows_c = np.where(att_valid, att_rows, 0)
        rows = np.concatenate([id_rows, att_rows_c])
        row_lists.append((id_rows, id_valid, att_rows_c, att_valid))

        hTo = np.ascontiguousarray(hT16[:, rows])  # [D, own] fp16
        im = {"hT": hT16, "hTo": hTo, "WT": WT16, "bb": bb}
        if nis:
            mT = np.zeros((N, rpad), dtype=np.int8)
            nval = int(att_valid.sum())
            if nval:
                mT[:, :nval] = adj8[att_rows_c[:nval], :].T
            im["mT"] = mT
        in_maps.append(im)

    def assemble(outs):
        out = np.empty((N, 256), dtype=np.float32)
        for c in range(NCORES):
            id_rows, id_valid, att_rows_c, att_valid = row_lists[c]
            o = outs[c]
            if id_valid.any():
                out[id_rows[id_valid]] = o[:kid][id_valid]
            if att_valid.any():
                out[att_rows_c[att_valid]] = o[kid:][att_valid]
        return out

    return nc, in_maps, assemble


def kernel(h, adj, W, b):
    nc, in_maps, assemble = _prepare(h, adj, W, b)

    from concourse.bass_utils import run_bass_kernel_spmd

    res = run_bass_kernel_spmd(nc, in_maps, core_ids=list(range(NCORES)))
    return assemble([res.results[c]["out"] for c in range(NCORES)])



# revision 6
# speedup vs baseline: 1.1628x; 1.1628x over previous
"""GAT layer kernel for Trainium2 (8 NeuronCores, SPMD, no collectives).

Math (reference):
    att = h @ h.T / sqrt(256)
    A = softmax(where(adj>0, att, -9e15), axis=1)
    A = (A + I) * 0.5; rows < k (k = nnz(adj[:,0])) overwritten with I
    out = relu(A @ (h @ W.T + b))

Algorithm (v2 — h-space flash attention, fp8 DoubleRow, diag extracted):
  Since softmax rows sum to 1 exactly, A @ (h W^T + b) = (A @ h) W^T + b,
  so the O(N^2) matmuls run in h-space and W is applied once at the end:
    rows [0,k):  out = relu(h @ W^T + b)
    rows [k,N):  out = relu((0.5*num/S + 0.5*h_i) @ W^T + b)
        num = num_off + r_i*h_i,  S = S_off + r_i
        num_off = sum_{j!=i} em[j,i]*h_j,   S_off = sum_{j!=i} em[j,i]
        em = exp(att/16 - 1.5) * mask_offdiag   (bias -1.5 recenters into
        fp8e4 range; it cancels in num/S)
        r_i = adj[i,i] * exp(|h_i|^2/16 - 1.5)  (host, f32 exact — the
        diagonal is the only entry that can overflow fp8, so it is zeroed
        in the mask and re-added exactly)
  Both big matmuls run in fp8e4 with perf_mode=DoubleRow (K=256 packed as
  [128,2,*]); exp is batched per jc-pair [128,1024] on ScalarE with fp8
  output; the mask multiply is one fp8 tensor_tensor per pair.
  Inputs stream in consumption order so compute starts ~1us in.

Sharding: identity rows and attention rows split evenly across 8 cores;
every core runs the same NEFF on different input slices.
"""

import math
import os
import sys

for _p in ("/opt/trn_rl_repo", "/root/.axon_site/_ro/trn_rl_repo"):
    if os.path.isdir(_p) and _p not in sys.path:
        sys.path.append(_p)

import numpy as np
import orjson

import concourse.bass as bass
import concourse.tile as tile
from concourse import mybir

F32 = mybir.dt.float32
F16 = mybir.dt.float16
BF16 = mybir.dt.bfloat16
F8 = mybir.dt.float8e4
DRMODE = mybir.MatmulPerfMode.DoubleRow

N = 8192
D = 256
NCORES = 8
NJC = N // 128  # 64 j-chunks of 128 rows
NJP = NJC // 2  # 32 j-pairs (DoubleRow K=256)
SCALE = 1.0 / 16.0
EBIAS = -1.5  # exp recentering; cancels in num/S


def _spill_waits(nc, max_sync=2):
    """Walrus rejects instructions with more sync commands than the lowered
    ISA struct can hold (2 for compute/DMA, 1 for NoOp/Drain). Tile can emit
    more. Move excess waits onto injected NoOps preceding the instruction
    (same engine, executes in order, so semantics are preserved)."""
    bir = orjson.loads(nc.to_json_bytes())
    for fn in bir["functions"]:
        for bb in fn["blocks"]:
            insts = bb.get("instructions") or []
            out = []
            for inst in insts:
                si = inst.get("sync_info")
                if si:
                    waits = si.get("on_wait") or []
                    upds = si.get("on_update") or []
                    lim = 1 if inst["opcode"] in ("NoOp", "Drain") else max_sync
                    cap = max(0, lim - len(upds))
                    if len(waits) > cap:
                        extra = waits[cap:]
                        si["on_wait"] = waits[:cap]
                        for ci, w in enumerate(extra):
                            out.append(
                                {
                                    "engine": inst["engine"],
                                    "ins": [],
                                    "outs": [],
                                    "name": f"{inst['name']}-sw{ci}",
                                    "opcode": "NoOp",
                                    "sync_info": {"on_wait": [w], "on_update": []},
                                    "debug": inst.get("debug", 0),
                                }
                            )
                out.append(inst)
            bb["instructions"] = out
    blob = orjson.dumps(bir)
    nc.to_json_bytes = lambda: blob


def _build(nid, nis, depth=2):
    """Build the SPMD program. nid/nis = number of 128-row identity /
    attention sub-tiles per core."""
    kid = nid * 128
    rpad = nis * 128
    own = kid + rpad
    ngroups = math.ceil(rpad / 512) if nis else 0

    nc = bass.Bass("TRN2", target_bir_lowering=False, debug=False, num_devices=NCORES)

    # --- dram inputs ---
    # id-rows phase (first: doubles as PE warmup)
    hIdT_d = nc.dram_tensor("hIdT", [D, kid], F16, kind="ExternalInput").ap()
    WT16_d = nc.dram_tensor("WT16", [D, 256], F16, kind="ExternalInput").ap()
    bb_d = nc.dram_tensor("bb", [128, 256], F32, kind="ExternalInput").ap()
    if nis:
        # fp8 DoubleRow operands, d-pair = (p, p+128)
        hTo8_d = nc.dram_tensor("hTo8", [128, 2 * rpad], F8, kind="ExternalInput").ap()
        hT8_d = nc.dram_tensor("hT8", [128, 2 * N], F8, kind="ExternalInput").ap()
        # mask (diag zeroed) [p, jc, i] and MM2 weights h rows [p, jc, d]
        m8_d = nc.dram_tensor("m8", [128, NJC * rpad], F8, kind="ExternalInput").ap()
        hpd_d = nc.dram_tensor("hpd", [128, NJC * 256], F8, kind="ExternalInput").ap()
        # end-phase: H = 0.5*h_att^T fp16 [d, i], rb = r broadcast f32 [128, i]
        H_d = nc.dram_tensor("H", [D, rpad], F16, kind="ExternalInput").ap()
        rb_d = nc.dram_tensor("rb", [128, rpad], F32, kind="ExternalInput").ap()
    out_d = nc.dram_tensor("out", [own, 256], F32, kind="ExternalOutput").ap()

    with tile.TileContext(nc) as tc:
        with (
            tc.tile_pool(name="big", bufs=1) as big,
            tc.tile_pool(name="work", bufs=3) as work,
            tc.tile_pool(name="fin", bufs=2) as fin,
            tc.tile_pool(name="att_ps", bufs=2, space="PSUM") as attp,
            tc.tile_pool(name="acc_ps", bufs=1, space="PSUM") as accp,
        ):
            # --- id-phase loads (small, first) ---
            hIdT_t = []
            WT16_t = []
            for dc in range(2):
                t = big.tile([128, kid], F16, tag=f"hIdT{dc}")
                nc.sync.dma_start(t[:], hIdT_d[dc * 128 : (dc + 1) * 128, :])
                hIdT_t.append(t)
                t = big.tile([128, 256], F16, tag=f"WT16{dc}")
                nc.sync.dma_start(t[:], WT16_d[dc * 128 : (dc + 1) * 128, :])
                WT16_t.append(t)
            bb_t = big.tile([128, 256], F32, tag="bb")
            nc.sync.dma_start(bb_t[:], bb_d[:, :])
            ebias_t = big.tile([128, 1], F32, tag="ebias")
            nc.vector.memset(ebias_t[:], EBIAS)

            # --- main-loop loads, in consumption order ---
            if nis:
                hTo8_t = big.tile([128, 2, rpad], F8, tag="hTo8")
                nc.sync.dma_start(
                    hTo8_t[:].rearrange("p v n -> p (v n)"), hTo8_d[:, :]
                )
                hT8_r = hT8_d.rearrange("p (v n) -> p v n", v=2)
                m8_r = m8_d.rearrange("p (a n) -> p a n", a=NJC)
                hpd_r = hpd_d.rearrange("p (a n) -> p a n", a=NJC)
                hT8_t = [None] * 4  # 16 jc per chunk
                m8_t = [None] * 8  # 8 jc per chunk
                hpd_t = [None] * 4  # 16 jc per chunk
                for c in range(4):
                    t = big.tile([128, 2, 2048], F8, tag=f"hT8_{c}")
                    nc.sync.dma_start(t[:], hT8_r[:, :, c * 2048 : (c + 1) * 2048])
                    hT8_t[c] = t
                    for mc in (2 * c, 2 * c + 1):
                        t = big.tile([128, 8, rpad], F8, tag=f"m8_{mc}")
                        nc.sync.dma_start(t[:], m8_r[:, mc * 8 : (mc + 1) * 8, :])
                        m8_t[mc] = t
                    t = big.tile([128, 16, 256], F8, tag=f"hpd_{c}")
                    nc.sync.dma_start(t[:], hpd_r[:, c * 16 : (c + 1) * 16, :])
                    hpd_t[c] = t
                # end-phase tensors (needed last)
                H_t = []
                for dc in range(2):
                    t = big.tile([128, rpad], F16, tag=f"H{dc}")
                    nc.sync.dma_start(t[:], H_d[dc * 128 : (dc + 1) * 128, :])
                    H_t.append(t)
                rb_t = big.tile([128, rpad], F32, tag="rb")
                nc.sync.dma_start(rb_t[:], rb_d[:, :])
                ones8 = big.tile([128, 2, 16], F8, tag="ones8")
                nc.vector.memset(ones8[:], 1.0)
                ones32 = big.tile([1, 128], F32, tag="ones32")
                nc.vector.memset(ones32[:], 1.0)

            # --- id phase: out rows [0,kid) = relu(h @ W^T + b) ---
            for it in range(nid):
                ps_t = accp.tile([128, 512], F32, tag=f"acc{it % 2}")
                ps = ps_t[:, 0:256]
                for dc in range(2):
                    nc.tensor.matmul(
                        ps,
                        hIdT_t[dc][:, it * 128 : (it + 1) * 128],
                        WT16_t[dc][:],
                        start=(dc == 0),
                        stop=(dc == 1),
                    )
                tmp = fin.tile([128, 256], F32, tag="id_tmp")
                nc.vector.tensor_tensor(tmp[:], ps, bb_t[:], op=mybir.AluOpType.add)
                o_t = fin.tile([128, 256], F32, tag="id_o")
                nc.vector.tensor_scalar_max(o_t[:], tmp[:], 0.0)
                nc.sync.dma_start(out_d[it * 128 : (it + 1) * 128, :], o_t[:])

            # --- attention phase ---
            for g in range(ngroups):
                i_lo = g * 512
                iw = min(512, rpad - i_lo)
                acc_d = [
                    accp.tile([128, 512], F32, tag=f"acc{dc}", name=f"acc{dc}")
                    for dc in range(2)
                ]
                s_ps = accp.tile([1, 512], F32, tag="s_ps")
                pend = []

                def emit_mm2(t, em_t, g=g, iw=iw, acc_d=acc_d, s_ps=s_ps):
                    for dc in range(2):
                        nc.tensor.matmul(
                            acc_d[dc][:, 0:iw],
                            hpd_t[t // 8][:, (t % 8) * 2 : (t % 8) * 2 + 2,
                                          dc * 128 : (dc + 1) * 128],
                            em_t[:, :, 0:iw],
                            start=(t == 0),
                            stop=(t == NJP - 1),
                            perf_mode=DRMODE,
                        )
                    nc.tensor.matmul(
                        s_ps[:, 0:iw],
                        ones8[:, :, 0:1],
                        em_t[:, :, 0:iw],
                        start=(t == 0),
                        stop=(t == NJP - 1),
                        perf_mode=DRMODE,
                    )

                for t in range(NJP):
                    att_ps = attp.tile([128, 1024], F32, tag="att")
                    for v in range(2):
                        jc = 2 * t + v
                        nc.tensor.matmul(
                            att_ps[:, v * 512 : v * 512 + iw],
                            hT8_t[jc // 16][:, :, (jc % 16) * 128 : (jc % 16 + 1) * 128],
                            hTo8_t[:, :, i_lo : i_lo + iw],
                            start=True,
                            stop=True,
                            perf_mode=DRMODE,
                        )
                    e8_t = work.tile([128, 1024], BF16, tag="e8")
                    nc.scalar.activation(
                        e8_t[:],
                        att_ps[:],
                        mybir.ActivationFunctionType.Exp,
                        scale=SCALE,
                        bias=ebias_t[:],
                    )
                    em_t = work.tile([128, 2, 512], F8, tag="em")
                    nc.vector.tensor_tensor(
                        em_t[:, :, :],
                        e8_t[:].rearrange("p (v n) -> p v n", v=2),
                        m8_t[t // 4][:, (t % 4) * 2 : (t % 4) * 2 + 2,
                                     i_lo : i_lo + 512],
                        op=mybir.AluOpType.mult,
                    )
                    pend.append((t, em_t))
                    if len(pend) > depth:
                        emit_mm2(*pend.pop(0))
                for item in pend:
                    emit_mm2(*item)

                # --- end phase for this group ---
                # S_full = S_off + r ; recipS broadcast via f32 ones matmul
                s_full = fin.tile([1, 512], F32, tag="s_full")
                nc.vector.tensor_tensor(
                    s_full[:, 0:iw], s_ps[:, 0:iw], rb_t[0:1, i_lo : i_lo + iw],
                    op=mybir.AluOpType.add,
                )
                s_rec = fin.tile([1, 512], F32, tag="s_rec")
                nc.vector.reciprocal(s_rec[:, 0:iw], s_full[:, 0:iw])
                rec_bc = attp.tile([128, 1024], F32, tag="att")
                nc.tensor.matmul(
                    rec_bc[:, 0:iw], ones32[:], s_rec[:, 0:iw], start=True, stop=True
                )
                blend = []
                for dc in range(2):
                    t0 = fin.tile([128, 512], F32, tag="bl_t0")
                    nc.vector.tensor_tensor(
                        t0[:, 0:iw],
                        rb_t[:, i_lo : i_lo + iw],
                        H_t[dc][:, i_lo : i_lo + iw],
                        op=mybir.AluOpType.mult,
                    )
                    t1 = fin.tile([128, 512], F32, tag="bl_t1")
                    nc.vector.scalar_tensor_tensor(
                        t1[:, 0:iw],
                        acc_d[dc][:, 0:iw],
                        0.5,
                        t0[:, 0:iw],
                        op0=mybir.AluOpType.mult,
                        op1=mybir.AluOpType.add,
                    )
                    t2 = fin.tile([128, 512], F32, tag="bl_t2")
                    nc.vector.tensor_tensor(
                        t2[:, 0:iw], t1[:, 0:iw], rec_bc[:, 0:iw],
                        op=mybir.AluOpType.mult,
                    )
                    bl = fin.tile([128, 512], F16, tag=f"blend{dc}")
                    nc.vector.tensor_tensor(
                        bl[:, 0:iw], t2[:, 0:iw], H_t[dc][:, i_lo : i_lo + iw],
                        op=mybir.AluOpType.add,
                    )
                    blend.append(bl)
                # W apply + bias + relu + store
                for it in range(iw // 128):
                    ps_t = accp.tile([128, 512], F32, tag=f"acc{it % 2}")
                    ps = ps_t[:, 0:256]
                    for dc in range(2):
                        nc.tensor.matmul(
                            ps,
                            blend[dc][:, it * 128 : (it + 1) * 128],
                            WT16_t[dc][:],
                            start=(dc == 0),
                            stop=(dc == 1),
                        )
                    tmp = fin.tile([128, 256], F32, tag="w_tmp")
                    nc.vector.tensor_tensor(
                        tmp[:], ps, bb_t[:], op=mybir.AluOpType.add
                    )
                    o_t = fin.tile([128, 256], F32, tag="w_o")
                    nc.vector.tensor_scalar_max(o_t[:], tmp[:], 0.0)
                    r0 = kid + i_lo + it * 128
                    nc.sync.dma_start(out_d[r0 : r0 + 128, :], o_t[:])

    _spill_waits(nc)
    return nc


_CACHE = {}


def _prepare(h, adj, W, b):
    """Host-side sharding + layout prep. Returns (nc, in_maps, assemble)."""
    h = np.asarray(h, dtype=np.float32)
    adj = np.asarray(adj)
    W = np.asarray(W, dtype=np.float32)
    b = np.asarray(b, dtype=np.float32)

    k = int(np.count_nonzero(adj[:, 0]))
    nid = (k + NCORES * 128 - 1) // (NCORES * 128)
    nis = (N - k + NCORES * 128 - 1) // (NCORES * 128)
    key = (nid, nis)
    if key not in _CACHE:
        _CACHE[key] = _build(nid, nis)
    nc = _CACHE[key]

    kid = nid * 128
    rpad = nis * 128

    f8 = mybir.dt.np(F8)
    adj8 = (adj != 0)
    # shared across cores
    h8 = h.astype(f8)  # [N, 256] fp8
    hT8 = np.ascontiguousarray(h8.T)  # [256, N]
    hT8_dr = hT8.reshape(2, 128, N).transpose(1, 0, 2).reshape(128, 2 * N)
    hT8_dr = np.ascontiguousarray(hT8_dr)
    # hpd[p, jc, d] = h8[jc*128 + p, d]
    hpd = np.ascontiguousarray(
        h8.reshape(NJC, 128, 256).transpose(1, 0, 2)
    ).reshape(128, NJC * 256)
    WT16 = np.ascontiguousarray(W.T).astype(np.float16)
    bb = np.broadcast_to(b, (128, 256)).astype(np.float32).copy()
    # diagonal r_i = adj_ii * exp(|h_i|^2 * SCALE + EBIAS)  (f32 exact)
    dot_ii = np.einsum("nd,nd->n", h, h)
    r_full = np.where(adj8.diagonal(), np.exp(dot_ii * SCALE + EBIAS), 0.0).astype(
        np.float32
    )

    hT32 = h.T  # [256, N] f32

    in_maps = []
    row_lists = []
    for c in range(NCORES):
        id_rows = np.arange(c * kid, (c + 1) * kid)
        id_valid = id_rows < k
        id_rows = np.where(id_valid, id_rows, 0)
        att_rows = np.arange(k + c * rpad, k + (c + 1) * rpad)
        att_valid = att_rows < N
        att_rows_c = np.where(att_valid, att_rows, 0)
        row_lists.append((id_rows, id_valid, att_rows_c, att_valid))

        im = {
            "hIdT": np.ascontiguousarray(hT32[:, id_rows]).astype(np.float16),
            "WT16": WT16,
            "bb": bb,
        }
        if nis:
            hTo8 = hT8[:, att_rows_c]  # [256, rpad]
            im["hTo8"] = np.ascontiguousarray(
                hTo8.reshape(2, 128, rpad).transpose(1, 0, 2)
            ).reshape(128, 2 * rpad)
            im["hT8"] = hT8_dr
            im["hpd"] = hpd
            # mask [p, jc, i] = adj[att_row_i, jc*128+p], diag zeroed
            mT = adj8[att_rows_c, :].T.astype(np.int8)  # [N, rpad]
            nval = int(att_valid.sum())
            if nval < rpad:
                mT[:, nval:] = 0
            mT[att_rows_c[:nval], np.arange(nval)] = 0  # zero diagonal
            m8 = np.ascontiguousarray(
                mT.reshape(NJC, 128, rpad).transpose(1, 0, 2)
            ).astype(f8).reshape(128, NJC * rpad)
            im["m8"] = m8
            im["H"] = np.ascontiguousarray(0.5 * hT32[:, att_rows_c]).astype(
                np.float16
            )
            r_c = np.where(att_valid, r_full[att_rows_c], 1.0).astype(np.float32)
            im["rb"] = np.ascontiguousarray(
                np.broadcast_to(r_c, (128, rpad))
            )
        in_maps.append(im)

    def assemble(outs):
        out = np.empty((N, 256), dtype=np.float32)
        for c in range(NCORES):
            id_rows, id_valid, att_rows_c, att_valid = row_lists[c]
            o = outs[c]
            if id_valid.any():
                out[id_rows[id_valid]] = o[:kid][id_valid]
            if att_valid.any():
                out[att_rows_c[att_valid]] = o[kid:][att_valid]
        return out

    return nc, in_maps, assemble


def kernel(h, adj, W, b):
    nc, in_maps, assemble = _prepare(h, adj, W, b)

    from concourse.bass_utils import run_bass_kernel_spmd

    res = run_bass_kernel_spmd(nc, in_maps, core_ids=list(range(NCORES)))
    return assemble([res.results[c]["out"] for c in range(NCORES)])


# revision 7
# speedup vs baseline: 1.1960x; 1.0285x over previous
"""GAT layer kernel for Trainium2 (8 NeuronCores, SPMD, no collectives).

Math (reference):
    att = h @ h.T / sqrt(256)
    A = softmax(where(adj>0, att, -9e15), axis=1)
    A = (A + I) * 0.5; rows < k (k = nnz(adj[:,0])) overwritten with I
    out = relu(A @ (h @ W.T + b))

Algorithm (v2 — h-space flash attention, fp8 DoubleRow, diag extracted):
  Since softmax rows sum to 1 exactly, A @ (h W^T + b) = (A @ h) W^T + b,
  so the O(N^2) matmuls run in h-space and W is applied once at the end:
    rows [0,k):  out = relu(h @ W^T + b)
    rows [k,N):  out = relu((0.5*num/S + 0.5*h_i) @ W^T + b)
        num = num_off + r_i*h_i,  S = S_off + r_i
        num_off = sum_{j!=i} em[j,i]*h_j,   S_off = sum_{j!=i} em[j,i]
        em = exp(att/16 - 1.5) * mask_offdiag   (bias -1.5 recenters into
        fp8e4 range; it cancels in num/S)
        r_i = adj[i,i] * exp(|h_i|^2/16 - 1.5)  (host, f32 exact — the
        diagonal is the only entry that can overflow fp8, so it is zeroed
        in the mask and re-added exactly)
  Both big matmuls run in fp8e4 with perf_mode=DoubleRow (K=256 packed as
  [128,2,*]); exp is batched per jc-pair [128,1024] on ScalarE with fp8
  output; the mask multiply is one fp8 tensor_tensor per pair.
  Inputs stream in consumption order so compute starts ~1us in.

Sharding: identity rows and attention rows split evenly across 8 cores;
every core runs the same NEFF on different input slices.
"""

import math
import os
import sys

for _p in ("/opt/trn_rl_repo", "/root/.axon_site/_ro/trn_rl_repo"):
    if os.path.isdir(_p) and _p not in sys.path:
        sys.path.append(_p)

import numpy as np
import orjson

import concourse.bass as bass
import concourse.tile as tile
from concourse import mybir

F32 = mybir.dt.float32
F16 = mybir.dt.float16
BF16 = mybir.dt.bfloat16
F8 = mybir.dt.float8e4
F8E5 = mybir.dt.float8e5
DRMODE = mybir.MatmulPerfMode.DoubleRow

N = 8192
D = 256
NCORES = 8
NJC = N // 128  # 64 j-chunks of 128 rows
NJP = NJC // 2  # 32 j-pairs (DoubleRow K=256)
SCALE = 1.0 / 16.0
EBIAS = -1.5  # exp recentering; cancels in num/S


def _spill_waits(nc, max_sync=2):
    """Walrus rejects instructions with more sync commands than the lowered
    ISA struct can hold (2 for compute/DMA, 1 for NoOp/Drain). Tile can emit
    more. Move excess waits onto injected NoOps preceding the instruction
    (same engine, executes in order, so semantics are preserved)."""
    bir = orjson.loads(nc.to_json_bytes())
    for fn in bir["functions"]:
        for bb in fn["blocks"]:
            insts = bb.get("instructions") or []
            out = []
            for inst in insts:
                si = inst.get("sync_info")
                if si:
                    waits = si.get("on_wait") or []
                    upds = si.get("on_update") or []
                    lim = 1 if inst["opcode"] in ("NoOp", "Drain") else max_sync
                    cap = max(0, lim - len(upds))
                    if len(waits) > cap:
                        extra = waits[cap:]
                        si["on_wait"] = waits[:cap]
                        for ci, w in enumerate(extra):
                            out.append(
                                {
                                    "engine": inst["engine"],
                                    "ins": [],
                                    "outs": [],
                                    "name": f"{inst['name']}-sw{ci}",
                                    "opcode": "NoOp",
                                    "sync_info": {"on_wait": [w], "on_update": []},
                                    "debug": inst.get("debug", 0),
                                }
                            )
                out.append(inst)
            bb["instructions"] = out
    blob = orjson.dumps(bir)
    nc.to_json_bytes = lambda: blob


def _build(nid, nis, depth=2):
    """Build the SPMD program. nid/nis = number of 128-row identity /
    attention sub-tiles per core."""
    kid = nid * 128
    rpad = nis * 128
    own = kid + rpad
    ngroups = math.ceil(rpad / 512) if nis else 0

    nc = bass.Bass("TRN2", target_bir_lowering=False, debug=False, num_devices=NCORES)

    # --- dram inputs ---
    # id-rows phase (first: doubles as PE warmup)
    hIdT_d = nc.dram_tensor("hIdT", [D, kid], F16, kind="ExternalInput").ap()
    WT16_d = nc.dram_tensor("WT16", [D, 256], F16, kind="ExternalInput").ap()
    bb_d = nc.dram_tensor("bb", [128, 256], F32, kind="ExternalInput").ap()
    if nis:
        # fp8 DoubleRow operands, d-pair = (p, p+128)
        hTo8_d = nc.dram_tensor("hTo8", [128, 2 * rpad], F8, kind="ExternalInput").ap()
        hT8_d = nc.dram_tensor("hT8", [128, 2 * N], F8, kind="ExternalInput").ap()
        # mask (diag zeroed) [p, jc, i] and MM2 weights h rows [p, jc, d]
        m8_d = nc.dram_tensor("m8", [128, NJC * rpad], F8, kind="ExternalInput").ap()
        hpd_d = nc.dram_tensor("hpd", [128, NJC * 256], F8, kind="ExternalInput").ap()
        # end-phase: H = 0.5*h_att^T fp16 [d, i], rb = r broadcast f32 [128, i]
        H_d = nc.dram_tensor("H", [D, rpad], F16, kind="ExternalInput").ap()
        rb_d = nc.dram_tensor("rb", [128, rpad], F32, kind="ExternalInput").ap()
    out_d = nc.dram_tensor("out", [own, 256], F32, kind="ExternalOutput").ap()

    with tile.TileContext(nc) as tc:
        with (
            tc.tile_pool(name="big", bufs=1) as big,
            tc.tile_pool(name="work", bufs=3) as work,
            tc.tile_pool(name="fin", bufs=2) as fin,
            tc.tile_pool(name="att_ps", bufs=2, space="PSUM") as attp,
            tc.tile_pool(name="acc_ps", bufs=1, space="PSUM") as accp,
        ):
            # --- id-phase loads (small, first) ---
            hIdT_t = []
            WT16_t = []
            for dc in range(2):
                t = big.tile([128, kid], F16, tag=f"hIdT{dc}")
                nc.sync.dma_start(t[:], hIdT_d[dc * 128 : (dc + 1) * 128, :])
                hIdT_t.append(t)
                t = big.tile([128, 256], F16, tag=f"WT16{dc}")
                nc.sync.dma_start(t[:], WT16_d[dc * 128 : (dc + 1) * 128, :])
                WT16_t.append(t)
            bb_t = big.tile([128, 256], F32, tag="bb")
            nc.sync.dma_start(bb_t[:], bb_d[:, :])
            ebias_t = big.tile([128, 1], F32, tag="ebias")
            nc.vector.memset(ebias_t[:], EBIAS)

            # --- main-loop loads, in consumption order ---
            if nis:
                hTo8_t = big.tile([128, 2, rpad], F8, tag="hTo8")
                nc.sync.dma_start(
                    hTo8_t[:].rearrange("p v n -> p (v n)"), hTo8_d[:, :]
                )
                hT8_r = hT8_d.rearrange("p (v n) -> p v n", v=2)
                m8_r = m8_d.rearrange("p (a n) -> p a n", a=NJC)
                hpd_r = hpd_d.rearrange("p (a n) -> p a n", a=NJC)
                hT8_t = [None] * 4  # 16 jc per chunk
                m8_t = [None] * 8  # 8 jc per chunk
                hpd_t = [None] * 4  # 16 jc per chunk
                for c in range(4):
                    t = big.tile([128, 2, 2048], F8, tag=f"hT8_{c}")
                    nc.sync.dma_start(t[:], hT8_r[:, :, c * 2048 : (c + 1) * 2048])
                    hT8_t[c] = t
                    for mc in (2 * c, 2 * c + 1):
                        t = big.tile([128, 8, rpad], F8, tag=f"m8_{mc}")
                        nc.sync.dma_start(t[:], m8_r[:, mc * 8 : (mc + 1) * 8, :])
                        m8_t[mc] = t
                    t = big.tile([128, 16, 256], F8, tag=f"hpd_{c}")
                    nc.sync.dma_start(t[:], hpd_r[:, c * 16 : (c + 1) * 16, :])
                    hpd_t[c] = t
                # end-phase tensors (needed last)
                H_t = []
                for dc in range(2):
                    t = big.tile([128, rpad], F16, tag=f"H{dc}")
                    nc.sync.dma_start(t[:], H_d[dc * 128 : (dc + 1) * 128, :])
                    H_t.append(t)
                rb_t = big.tile([128, rpad], F32, tag="rb")
                nc.sync.dma_start(rb_t[:], rb_d[:, :])
                ones8 = big.tile([128, 2, 16], F8, tag="ones8")
                nc.vector.memset(ones8[:], 1.0)
                ones32 = big.tile([1, 128], F32, tag="ones32")
                nc.vector.memset(ones32[:], 1.0)

            # --- id phase: out rows [0,kid) = relu(h @ W^T + b) ---
            for it in range(nid):
                ps_t = accp.tile([128, 512], F32, tag=f"acc{it % 2}")
                ps = ps_t[:, 0:256]
                for dc in range(2):
                    nc.tensor.matmul(
                        ps,
                        hIdT_t[dc][:, it * 128 : (it + 1) * 128],
                        WT16_t[dc][:],
                        start=(dc == 0),
                        stop=(dc == 1),
                    )
                tmp = fin.tile([128, 256], F32, tag="id_tmp")
                nc.vector.tensor_tensor(tmp[:], ps, bb_t[:], op=mybir.AluOpType.add)
                o_t = fin.tile([128, 256], F32, tag="id_o")
                nc.vector.tensor_scalar_max(o_t[:], tmp[:], 0.0)
                nc.sync.dma_start(out_d[it * 128 : (it + 1) * 128, :], o_t[:])

            # --- attention phase ---
            for g in range(ngroups):
                i_lo = g * 512
                iw = min(512, rpad - i_lo)
                acc_d = [
                    accp.tile([128, 512], F32, tag=f"acc{dc}", name=f"acc{dc}")
                    for dc in range(2)
                ]
                s_ps = accp.tile([1, 512], F32, tag="s_ps")
                pend = []

                def emit_mm2(t, em_t, g=g, iw=iw, acc_d=acc_d, s_ps=s_ps):
                    for dc in range(2):
                        nc.tensor.matmul(
                            acc_d[dc][:, 0:iw],
                            hpd_t[t // 8][:, (t % 8) * 2 : (t % 8) * 2 + 2,
                                          dc * 128 : (dc + 1) * 128],
                            em_t[:, :, 0:iw],
                            start=(t == 0),
                            stop=(t == NJP - 1),
                            perf_mode=DRMODE,
                        )
                    nc.tensor.matmul(
                        s_ps[:, 0:iw],
                        ones8[:, :, 0:1],
                        em_t[:, :, 0:iw],
                        start=(t == 0),
                        stop=(t == NJP - 1),
                        perf_mode=DRMODE,
                    )

                for t in range(NJP):
                    att_ps = attp.tile([128, 1024], F32, tag="att")
                    for v in range(2):
                        jc = 2 * t + v
                        nc.tensor.matmul(
                            att_ps[:, v * 512 : v * 512 + iw],
                            hT8_t[jc // 16][:, :, (jc % 16) * 128 : (jc % 16 + 1) * 128],
                            hTo8_t[:, :, i_lo : i_lo + iw],
                            start=True,
                            stop=True,
                            perf_mode=DRMODE,
                        )
                    e8_t = work.tile([128, 1024], BF16, tag="e8")
                    nc.scalar.activation(
                        e8_t[:],
                        att_ps[:],
                        mybir.ActivationFunctionType.Exp,
                        scale=SCALE,
                        bias=ebias_t[:],
                    )
                    em_t = work.tile([128, 2, 512], F8E5, tag="em")
                    nc.vector.tensor_tensor(
                        em_t[:, :, :],
                        e8_t[:].rearrange("p (v n) -> p v n", v=2),
                        m8_t[t // 4][:, (t % 4) * 2 : (t % 4) * 2 + 2,
                                     i_lo : i_lo + 512],
                        op=mybir.AluOpType.mult,
                    )
                    pend.append((t, em_t))
                    if len(pend) > depth:
                        emit_mm2(*pend.pop(0))
                for item in pend:
                    emit_mm2(*item)

                # --- end phase for this group ---
                # S_full = S_off + r ; recipS broadcast via f32 ones matmul
                s_full = fin.tile([1, 512], F32, tag="s_full")
                nc.vector.tensor_tensor(
                    s_full[:, 0:iw], s_ps[:, 0:iw], rb_t[0:1, i_lo : i_lo + iw],
                    op=mybir.AluOpType.add,
                )
                s_rec = fin.tile([1, 512], F32, tag="s_rec")
                nc.vector.reciprocal(s_rec[:, 0:iw], s_full[:, 0:iw])
                rec_bc = attp.tile([128, 1024], F32, tag="att")
                nc.tensor.matmul(
                    rec_bc[:, 0:iw], ones32[:], s_rec[:, 0:iw], start=True, stop=True
                )
                blend = []
                for dc in range(2):
                    t0 = fin.tile([128, 512], F32, tag="bl_t0")
                    nc.vector.tensor_tensor(
                        t0[:, 0:iw],
                        rb_t[:, i_lo : i_lo + iw],
                        H_t[dc][:, i_lo : i_lo + iw],
                        op=mybir.AluOpType.mult,
                    )
                    t1 = fin.tile([128, 512], F32, tag="bl_t1")
                    nc.vector.scalar_tensor_tensor(
                        t1[:, 0:iw],
                        acc_d[dc][:, 0:iw],
                        0.5,
                        t0[:, 0:iw],
                        op0=mybir.AluOpType.mult,
                        op1=mybir.AluOpType.add,
                    )
                    t2 = fin.tile([128, 512], F32, tag="bl_t2")
                    nc.vector.tensor_tensor(
                        t2[:, 0:iw], t1[:, 0:iw], rec_bc[:, 0:iw],
                        op=mybir.AluOpType.mult,
                    )
                    bl = fin.tile([128, 512], F16, tag=f"blend{dc}")
                    nc.vector.tensor_tensor(
                        bl[:, 0:iw], t2[:, 0:iw], H_t[dc][:, i_lo : i_lo + iw],
                        op=mybir.AluOpType.add,
                    )
                    blend.append(bl)
                # W apply + bias + relu + store
                for it in range(iw // 128):
                    ps_t = accp.tile([128, 512], F32, tag=f"acc{it % 2}")
                    ps = ps_t[:, 0:256]
                    for dc in range(2):
                        nc.tensor.matmul(
                            ps,
                            blend[dc][:, it * 128 : (it + 1) * 128],
                            WT16_t[dc][:],
                            start=(dc == 0),
                            stop=(dc == 1),
                        )
                    tmp = fin.tile([128, 256], F32, tag="w_tmp")
                    nc.vector.tensor_tensor(
                        tmp[:], ps, bb_t[:], op=mybir.AluOpType.add
                    )
                    o_t = fin.tile([128, 256], F32, tag="w_o")
                    nc.vector.tensor_scalar_max(o_t[:], tmp[:], 0.0)
                    r0 = kid + i_lo + it * 128
                    nc.sync.dma_start(out_d[r0 : r0 + 128, :], o_t[:])

    _spill_waits(nc)
    return nc


_CACHE = {}


def _prepare(h, adj, W, b):
    """Host-side sharding + layout prep. Returns (nc, in_maps, assemble)."""
    h = np.asarray(h, dtype=np.float32)
    adj = np.asarray(adj)
    W = np.asarray(W, dtype=np.float32)
    b = np.asarray(b, dtype=np.float32)

    k = int(np.count_nonzero(adj[:, 0]))
    nid = (k + NCORES * 128 - 1) // (NCORES * 128)
    nis = (N - k + NCORES * 128 - 1) // (NCORES * 128)
    key = (nid, nis)
    if key not in _CACHE:
        _CACHE[key] = _build(nid, nis)
    nc = _CACHE[key]

    kid = nid * 128
    rpad = nis * 128

    f8 = mybir.dt.np(F8)
    adj8 = (adj != 0)
    # shared across cores
    h8 = h.astype(f8)  # [N, 256] fp8
    hT8 = np.ascontiguousarray(h8.T)  # [256, N]
    hT8_dr = hT8.reshape(2, 128, N).transpose(1, 0, 2).reshape(128, 2 * N)
    hT8_dr = np.ascontiguousarray(hT8_dr)
    # hpd[p, jc, d] = h8[jc*128 + p, d]
    hpd = np.ascontiguousarray(
        h8.reshape(NJC, 128, 256).transpose(1, 0, 2)
    ).reshape(128, NJC * 256)
    WT16 = np.ascontiguousarray(W.T).astype(np.float16)
    bb = np.broadcast_to(b, (128, 256)).astype(np.float32).copy()
    # diagonal r_i = adj_ii * exp(|h_i|^2 * SCALE + EBIAS)  (f32 exact)
    dot_ii = np.einsum("nd,nd->n", h, h)
    r_full = np.where(adj8.diagonal(), np.exp(dot_ii * SCALE + EBIAS), 0.0).astype(
        np.float32
    )

    hT32 = h.T  # [256, N] f32

    in_maps = []
    row_lists = []
    for c in range(NCORES):
        id_rows = np.arange(c * kid, (c + 1) * kid)
        id_valid = id_rows < k
        id_rows = np.where(id_valid, id_rows, 0)
        att_rows = np.arange(k + c * rpad, k + (c + 1) * rpad)
        att_valid = att_rows < N
        att_rows_c = np.where(att_valid, att_rows, 0)
        row_lists.append((id_rows, id_valid, att_rows_c, att_valid))

        im = {
            "hIdT": np.ascontiguousarray(hT32[:, id_rows]).astype(np.float16),
            "WT16": WT16,
            "bb": bb,
        }
        if nis:
            hTo8 = hT8[:, att_rows_c]  # [256, rpad]
            im["hTo8"] = np.ascontiguousarray(
                hTo8.reshape(2, 128, rpad).transpose(1, 0, 2)
            ).reshape(128, 2 * rpad)
            im["hT8"] = hT8_dr
            im["hpd"] = hpd
            # mask [p, jc, i] = adj[att_row_i, jc*128+p], diag zeroed
            mT = adj8[att_rows_c, :].T.astype(np.int8)  # [N, rpad]
            nval = int(att_valid.sum())
            if nval < rpad:
                mT[:, nval:] = 0
            mT[att_rows_c[:nval], np.arange(nval)] = 0  # zero diagonal
            m8 = np.ascontiguousarray(
                mT.reshape(NJC, 128, rpad).transpose(1, 0, 2)
            ).astype(f8).reshape(128, NJC * rpad)
            im["m8"] = m8
            im["H"] = np.ascontiguousarray(0.5 * hT32[:, att_rows_c]).astype(
                np.float16
            )
            r_c = np.where(att_valid, r_full[att_rows_c], 1.0).astype(np.float32)
            im["rb"] = np.ascontiguousarray(
                np.broadcast_to(r_c, (128, rpad))
            )
        in_maps.append(im)

    def assemble(outs):
        out = np.empty((N, 256), dtype=np.float32)
        for c in range(NCORES):
            id_rows, id_valid, att_rows_c, att_valid = row_lists[c]
            o = outs[c]
            if id_valid.any():
                out[id_rows[id_valid]] = o[:kid][id_valid]
            if att_valid.any():
                out[att_rows_c[att_valid]] = o[kid:][att_valid]
        return out

    return nc, in_maps, assemble


def kernel(h, adj, W, b):
    nc, in_maps, assemble = _prepare(h, adj, W, b)

    from concourse.bass_utils import run_bass_kernel_spmd

    res = run_bass_kernel_spmd(nc, in_maps, core_ids=list(range(NCORES)))
    return assemble([res.results[c]["out"] for c in range(NCORES)])


# revision 9
# speedup vs baseline: 1.1991x; 1.0027x over previous
"""GAT layer kernel for Trainium2 (8 NeuronCores, SPMD, no collectives).

Math (reference):
    att = h @ h.T / sqrt(256)
    A = softmax(where(adj>0, att, -9e15), axis=1)
    A = (A + I) * 0.5; rows < k (k = nnz(adj[:,0])) overwritten with I
    out = relu(A @ (h @ W.T + b))

Algorithm (v2 — h-space flash attention, fp8 DoubleRow, diag extracted):
  Since softmax rows sum to 1 exactly, A @ (h W^T + b) = (A @ h) W^T + b,
  so the O(N^2) matmuls run in h-space and W is applied once at the end:
    rows [0,k):  out = relu(h @ W^T + b)
    rows [k,N):  out = relu((0.5*num/S + 0.5*h_i) @ W^T + b)
        num = num_off + r_i*h_i,  S = S_off + r_i
        num_off = sum_{j!=i} em[j,i]*h_j,   S_off = sum_{j!=i} em[j,i]
        em = exp(att/16 - 1.5) * mask_offdiag   (bias -1.5 recenters into
        fp8e4 range; it cancels in num/S)
        r_i = adj[i,i] * exp(|h_i|^2/16 - 1.5)  (host, f32 exact — the
        diagonal is the only entry that can overflow fp8, so it is zeroed
        in the mask and re-added exactly)
  Both big matmuls run in fp8e4 with perf_mode=DoubleRow (K=256 packed as
  [128,2,*]); exp is batched per jc-pair [128,1024] on ScalarE with fp8
  output; the mask multiply is one fp8 tensor_tensor per pair.
  Inputs stream in consumption order so compute starts ~1us in.

Sharding: identity rows and attention rows split evenly across 8 cores;
every core runs the same NEFF on different input slices.
"""

import math
import os
import sys

for _p in ("/opt/trn_rl_repo", "/root/.axon_site/_ro/trn_rl_repo"):
    if os.path.isdir(_p) and _p not in sys.path:
        sys.path.append(_p)

import numpy as np
import orjson

import concourse.bass as bass
import concourse.tile as tile
from concourse import mybir

F32 = mybir.dt.float32
F16 = mybir.dt.float16
BF16 = mybir.dt.bfloat16
F8 = mybir.dt.float8e4
F8E5 = mybir.dt.float8e5
I8 = mybir.dt.int8
DRMODE = mybir.MatmulPerfMode.DoubleRow

N = 8192
D = 256
NCORES = 8
NJC = N // 128  # 64 j-chunks of 128 rows
NJP = NJC // 2  # 32 j-pairs (DoubleRow K=256)
SCALE = 1.0 / 16.0
EBIAS = -1.5  # exp recentering; cancels in num/S


def _spill_waits(nc, max_sync=2):
    """Walrus rejects instructions with more sync commands than the lowered
    ISA struct can hold (2 for compute/DMA, 1 for NoOp/Drain). Tile can emit
    more. Move excess waits onto injected NoOps preceding the instruction
    (same engine, executes in order, so semantics are preserved)."""
    bir = orjson.loads(nc.to_json_bytes())
    for fn in bir["functions"]:
        for bb in fn["blocks"]:
            insts = bb.get("instructions") or []
            out = []
            for inst in insts:
                si = inst.get("sync_info")
                if si:
                    waits = si.get("on_wait") or []
                    upds = si.get("on_update") or []
                    lim = 1 if inst["opcode"] in ("NoOp", "Drain") else max_sync
                    cap = max(0, lim - len(upds))
                    if len(waits) > cap:
                        extra = waits[cap:]
                        si["on_wait"] = waits[:cap]
                        for ci, w in enumerate(extra):
                            out.append(
                                {
                                    "engine": inst["engine"],
                                    "ins": [],
                                    "outs": [],
                                    "name": f"{inst['name']}-sw{ci}",
                                    "opcode": "NoOp",
                                    "sync_info": {"on_wait": [w], "on_update": []},
                                    "debug": inst.get("debug", 0),
                                }
                            )
                out.append(inst)
            bb["instructions"] = out
    blob = orjson.dumps(bir)
    nc.to_json_bytes = lambda: blob


def _build(nid, nis, depth=2):
    """Build the SPMD program. nid/nis = number of 128-row identity /
    attention sub-tiles per core."""
    kid = nid * 128
    rpad = nis * 128
    own = kid + rpad
    ngroups = math.ceil(rpad / 512) if nis else 0

    nc = bass.Bass("TRN2", target_bir_lowering=False, debug=False, num_devices=NCORES)

    # --- dram inputs ---
    # id-rows phase (first: doubles as PE warmup)
    hIdT_d = nc.dram_tensor("hIdT", [D, kid], F16, kind="ExternalInput").ap()
    WT16_d = nc.dram_tensor("WT16", [D, 256], F16, kind="ExternalInput").ap()
    bb_d = nc.dram_tensor("bb", [128, 256], F32, kind="ExternalInput").ap()
    if nis:
        # fp8 DoubleRow operands, d-pair = (p, p+128)
        hTo8_d = nc.dram_tensor("hTo8", [128, 2 * rpad], F8, kind="ExternalInput").ap()
        hT8_d = nc.dram_tensor("hT8", [128, 2 * N], F8, kind="ExternalInput").ap()
        # mask (diag zeroed) [p, jc, i] and MM2 weights h rows [p, jc, d]
        m8_d = nc.dram_tensor("m8", [128, NJC * rpad], I8, kind="ExternalInput").ap()
        hpd_d = nc.dram_tensor("hpd", [128, NJC * 256], F8, kind="ExternalInput").ap()
        # end-phase: H = 0.5*h_att^T fp16 [d, i], rb = r broadcast f32 [128, i]
        H_d = nc.dram_tensor("H", [D, rpad], F16, kind="ExternalInput").ap()
        rb_d = nc.dram_tensor("rb", [128, rpad], F32, kind="ExternalInput").ap()
    out_d = nc.dram_tensor("out", [own, 256], F32, kind="ExternalOutput").ap()

    with tile.TileContext(nc) as tc:
        with (
            tc.tile_pool(name="big", bufs=1) as big,
            tc.tile_pool(name="work", bufs=3) as work,
            tc.tile_pool(name="fin", bufs=2) as fin,
            tc.tile_pool(name="att_ps", bufs=2, space="PSUM") as attp,
            tc.tile_pool(name="acc_ps", bufs=1, space="PSUM") as accp,
        ):
            # --- id-phase loads (small, first) ---
            hIdT_t = []
            WT16_t = []
            for dc in range(2):
                t = big.tile([128, kid], F16, tag=f"hIdT{dc}")
                nc.sync.dma_start(t[:], hIdT_d[dc * 128 : (dc + 1) * 128, :])
                hIdT_t.append(t)
                t = big.tile([128, 256], F16, tag=f"WT16{dc}")
                nc.sync.dma_start(t[:], WT16_d[dc * 128 : (dc + 1) * 128, :])
                WT16_t.append(t)
            bb_t = big.tile([128, 256], F32, tag="bb")
            nc.sync.dma_start(bb_t[:], bb_d[:, :])
            ebias_t = big.tile([128, 1], F32, tag="ebias")
            nc.vector.memset(ebias_t[:], EBIAS)

            # --- main-loop loads, in consumption order ---
            if nis:
                hTo8_t = big.tile([128, 2, rpad], F8, tag="hTo8")
                nc.sync.dma_start(
                    hTo8_t[:].rearrange("p v n -> p (v n)"), hTo8_d[:, :]
                )
                H_t = []
                for dc in range(2):
                    t = big.tile([128, rpad], F16, tag=f"H{dc}")
                    nc.sync.dma_start(t[:], H_d[dc * 128 : (dc + 1) * 128, :])
                    H_t.append(t)
                rb_t = big.tile([128, rpad], F32, tag="rb")
                nc.sync.dma_start(rb_t[:], rb_d[:, :])
                hT8_r = hT8_d.rearrange("p (v n) -> p v n", v=2)
                m8_r = m8_d.rearrange("p (a n) -> p a n", a=NJC)
                hpd_r = hpd_d.rearrange("p (a n) -> p a n", a=NJC)
                hT8_t = [None] * 4  # 16 jc per chunk
                m8_t = [None] * 8  # 8 jc per chunk
                hpd_t = [None] * 4  # 16 jc per chunk
                for c in range(4):
                    t = big.tile([128, 2, 2048], F8, tag=f"hT8_{c}")
                    nc.sync.dma_start(t[:], hT8_r[:, :, c * 2048 : (c + 1) * 2048])
                    hT8_t[c] = t
                    for mc in (2 * c, 2 * c + 1):
                        t = big.tile([128, 8, rpad], I8, tag=f"m8_{mc}")
                        nc.sync.dma_start(t[:], m8_r[:, mc * 8 : (mc + 1) * 8, :])
                        m8_t[mc] = t
                    t = big.tile([128, 16, 256], F8, tag=f"hpd_{c}")
                    nc.sync.dma_start(t[:], hpd_r[:, c * 16 : (c + 1) * 16, :])
                    hpd_t[c] = t
                ones8 = big.tile([128, 2, 16], F8, tag="ones8")
                nc.vector.memset(ones8[:], 1.0)
                ones32 = big.tile([1, 128], F32, tag="ones32")
                nc.vector.memset(ones32[:], 1.0)
                # E: PE warmup - DMA-independent dummy matmuls to engage HAM
                wz = big.tile([128, 2, 512], F8, tag="wz")
                nc.vector.memset(wz[:].rearrange("p v n -> p (v n)"), 0.0)
                warm_ps = accp.tile([128, 512], F32, tag="warm")
                for _w in range(10):
                    nc.tensor.matmul(
                        warm_ps[0:16, :], ones8[:, :, 0:16], wz[:],
                        start=True, stop=True, perf_mode=DRMODE,
                        skip_group_check=True,
                    )

            # --- id phase: out rows [0,kid) = relu(h @ W^T + b) ---
            for it in range(nid):
                ps_t = accp.tile([128, 512], F32, tag=f"acc{it % 2}")
                ps = ps_t[:, 0:256]
                for dc in range(2):
                    nc.tensor.matmul(
                        ps,
                        hIdT_t[dc][:, it * 128 : (it + 1) * 128],
                        WT16_t[dc][:],
                        start=(dc == 0),
                        stop=(dc == 1),
                    )
                tmp = fin.tile([128, 256], F32, tag="id_tmp")
                nc.vector.tensor_tensor(tmp[:], ps, bb_t[:], op=mybir.AluOpType.add)
                o_t = fin.tile([128, 256], F32, tag="id_o")
                nc.vector.tensor_scalar_max(o_t[:], tmp[:], 0.0)
                nc.sync.dma_start(out_d[it * 128 : (it + 1) * 128, :], o_t[:])

            # --- hoisted loop-invariant: t0 = rb * H (diag term) ---
            t0_t = []
            if nis:
                for dc in range(2):
                    t0 = big.tile([128, rpad], F32, tag=f"t0_{dc}")
                    nc.vector.tensor_tensor(
                        t0[:], rb_t[:], H_t[dc][:], op=mybir.AluOpType.mult
                    )
                    t0_t.append(t0)

            # --- attention phase ---
            for g in range(ngroups):
                i_lo = g * 512
                iw = min(512, rpad - i_lo)
                acc_d = [
                    accp.tile([128, 512], F32, tag=f"acc{dc}", name=f"acc{dc}")
                    for dc in range(2)
                ]
                s_ps = accp.tile([16, 512], F32, tag="s_ps")
                pend = []

                def emit_mm2(t, em_t, g=g, iw=iw, acc_d=acc_d, s_ps=s_ps):
                    for dc in range(2):
                        nc.tensor.matmul(
                            acc_d[dc][:, 0:iw],
                            hpd_t[t // 8][:, (t % 8) * 2 : (t % 8) * 2 + 2,
                                          dc * 128 : (dc + 1) * 128],
                            em_t[:, :, 0:iw],
                            start=(t == 0),
                            stop=(t == NJP - 1),
                            perf_mode=DRMODE,
                        )
                    nc.tensor.matmul(
                        s_ps[:, 0:iw],
                        ones8[:, :, 0:16],
                        em_t[:, :, 0:iw],
                        start=(t == 0),
                        stop=(t == NJP - 1),
                        perf_mode=DRMODE,
                    )

                for t in range(NJP):
                    att_ps = attp.tile([128, 1024], F32, tag="att")
                    for v in range(2):
                        jc = 2 * t + v
                        nc.tensor.matmul(
                            att_ps[:, v * 512 : v * 512 + iw],
                            hT8_t[jc // 16][:, :, (jc % 16) * 128 : (jc % 16 + 1) * 128],
                            hTo8_t[:, :, i_lo : i_lo + iw],
                            start=True,
                            stop=True,
                            perf_mode=DRMODE,
                        )
                    e8_t = work.tile([128, 1024], F8E5, tag="e8")
                    nc.scalar.activation(
                        e8_t[:],
                        att_ps[:],
                        mybir.ActivationFunctionType.Exp,
                        scale=SCALE,
                        bias=ebias_t[:],
                    )
                    em_t = work.tile([128, 2, 512], F8E5, tag="em")
                    nc.vector.tensor_tensor(
                        em_t[:, :, :].bitcast(I8),
                        e8_t[:].rearrange("p (v n) -> p v n", v=2).bitcast(I8),
                        m8_t[t // 4][:, (t % 4) * 2 : (t % 4) * 2 + 2,
                                     i_lo : i_lo + 512],
                        op=mybir.AluOpType.bitwise_and,
                    )
                    pend.append((t, em_t))
                    if len(pend) > depth:
                        emit_mm2(*pend.pop(0))
                for item in pend:
                    emit_mm2(*item)

                # --- end phase for this group ---
                # S_full = S_off + r ; recipS broadcast via f32 ones matmul
                s_full = fin.tile([16, 512], F32, tag="s_full")
                nc.vector.tensor_tensor(
                    s_full[:, 0:iw], s_ps[:, 0:iw], rb_t[0:16, i_lo : i_lo + iw],
                    op=mybir.AluOpType.add,
                )
                s_rec = fin.tile([16, 512], F32, tag="s_rec")
                nc.vector.reciprocal(s_rec[:, 0:iw], s_full[:, 0:iw])
                rec_bc = attp.tile([128, 1024], F32, tag="att")
                nc.tensor.matmul(
                    rec_bc[:, 0:iw], ones32[:], s_rec[0:1, 0:iw], start=True,
                    stop=True,
                )
                blend = []
                for dc in range(2):
                    t1 = fin.tile([128, 512], F32, tag="bl_t1")
                    nc.vector.scalar_tensor_tensor(
                        t1[:, 0:iw],
                        acc_d[dc][:, 0:iw],
                        0.5,
                        t0_t[dc][:, i_lo : i_lo + iw],
                        op0=mybir.AluOpType.mult,
                        op1=mybir.AluOpType.add,
                    )
                    t2 = fin.tile([128, 512], F32, tag="bl_t2")
                    nc.vector.tensor_tensor(
                        t2[:, 0:iw], t1[:, 0:iw], rec_bc[:, 0:iw],
                        op=mybir.AluOpType.mult,
                    )
                    bl = fin.tile([128, 512], F16, tag=f"blend{dc}")
                    nc.vector.tensor_tensor(
                        bl[:, 0:iw], t2[:, 0:iw], H_t[dc][:, i_lo : i_lo + iw],
                        op=mybir.AluOpType.add,
                    )
                    blend.append(bl)
                # W apply + bias + relu + store
                for it in range(iw // 128):
                    ps_t = accp.tile([128, 512], F32, tag=f"acc{it % 2}")
                    ps = ps_t[:, 0:256]
                    for dc in range(2):
                        nc.tensor.matmul(
                            ps,
                            blend[dc][:, it * 128 : (it + 1) * 128],
                            WT16_t[dc][:],
                            start=(dc == 0),
                            stop=(dc == 1),
                        )
                    tmp = fin.tile([128, 256], F32, tag="w_tmp")
                    nc.vector.tensor_tensor(
                        tmp[:], ps, bb_t[:], op=mybir.AluOpType.add
                    )
                    o_t = fin.tile([128, 256], F32, tag="w_o")
                    nc.vector.tensor_scalar_max(o_t[:], tmp[:], 0.0)
                    r0 = kid + i_lo + it * 128
                    nc.sync.dma_start(out_d[r0 : r0 + 128, :], o_t[:])

    _spill_waits(nc)
    return nc


_CACHE = {}


def _prepare(h, adj, W, b):
    """Host-side sharding + layout prep. Returns (nc, in_maps, assemble)."""
    h = np.asarray(h, dtype=np.float32)
    adj = np.asarray(adj)
    W = np.asarray(W, dtype=np.float32)
    b = np.asarray(b, dtype=np.float32)

    k = int(np.count_nonzero(adj[:, 0]))
    nid = (k + NCORES * 128 - 1) // (NCORES * 128)
    nis = (N - k + NCORES * 128 - 1) // (NCORES * 128)
    key = (nid, nis)
    if key not in _CACHE:
        _CACHE[key] = _build(nid, nis)
    nc = _CACHE[key]

    kid = nid * 128
    rpad = nis * 128

    f8 = mybir.dt.np(F8)
    adj8 = (adj != 0)
    # shared across cores
    h8 = h.astype(f8)  # [N, 256] fp8
    hT8 = np.ascontiguousarray(h8.T)  # [256, N]
    hT8_dr = hT8.reshape(2, 128, N).transpose(1, 0, 2).reshape(128, 2 * N)
    hT8_dr = np.ascontiguousarray(hT8_dr)
    # hpd[p, jc, d] = h8[jc*128 + p, d]
    hpd = np.ascontiguousarray(
        h8.reshape(NJC, 128, 256).transpose(1, 0, 2)
    ).reshape(128, NJC * 256)
    WT16 = np.ascontiguousarray(W.T).astype(np.float16)
    bb = np.broadcast_to(b, (128, 256)).astype(np.float32).copy()
    # diagonal r_i = adj_ii * exp(|h_i|^2 * SCALE + EBIAS)  (f32 exact)
    dot_ii = np.einsum("nd,nd->n", h, h)
    r_full = np.where(adj8.diagonal(), np.exp(dot_ii * SCALE + EBIAS), 0.0).astype(
        np.float32
    )

    hT32 = h.T  # [256, N] f32

    in_maps = []
    row_lists = []
    for c in range(NCORES):
        id_rows = np.arange(c * kid, (c + 1) * kid)
        id_valid = id_rows < k
        id_rows = np.where(id_valid, id_rows, 0)
        att_rows = np.arange(k + c * rpad, k + (c + 1) * rpad)
        att_valid = att_rows < N
        att_rows_c = np.where(att_valid, att_rows, 0)
        row_lists.append((id_rows, id_valid, att_rows_c, att_valid))

        im = {
            "hIdT": np.ascontiguousarray(hT32[:, id_rows]).astype(np.float16),
            "WT16": WT16,
            "bb": bb,
        }
        if nis:
            hTo8 = hT8[:, att_rows_c]  # [256, rpad]
            im["hTo8"] = np.ascontiguousarray(
                hTo8.reshape(2, 128, rpad).transpose(1, 0, 2)
            ).reshape(128, 2 * rpad)
            im["hT8"] = hT8_dr
            im["hpd"] = hpd
            # mask [p, jc, i] = adj[att_row_i, jc*128+p], diag zeroed
            mT = adj8[att_rows_c, :].T.astype(np.int8)  # [N, rpad]
            nval = int(att_valid.sum())
            if nval < rpad:
                mT[:, nval:] = 0
            mT[att_rows_c[:nval], np.arange(nval)] = 0  # zero diagonal
            m8 = (
                np.ascontiguousarray(
                    mT.reshape(NJC, 128, rpad).transpose(1, 0, 2)
                )
                * np.int8(-1)
            ).reshape(128, NJC * rpad)
            im["m8"] = m8
            im["H"] = np.ascontiguousarray(0.5 * hT32[:, att_rows_c]).astype(
                np.float16
            )
            r_c = np.where(att_valid, r_full[att_rows_c], 1.0).astype(np.float32)
            im["rb"] = np.ascontiguousarray(
                np.broadcast_to(r_c, (128, rpad))
            )
        in_maps.append(im)

    def assemble(outs):
        out = np.empty((N, 256), dtype=np.float32)
        for c in range(NCORES):
            id_rows, id_valid, att_rows_c, att_valid = row_lists[c]
            o = outs[c]
            if id_valid.any():
                out[id_rows[id_valid]] = o[:kid][id_valid]
            if att_valid.any():
                out[att_rows_c[att_valid]] = o[kid:][att_valid]
        return out

    return nc, in_maps, assemble


def kernel(h, adj, W, b):
    nc, in_maps, assemble = _prepare(h, adj, W, b)

    from concourse.bass_utils import run_bass_kernel_spmd

    res = run_bass_kernel_spmd(nc, in_maps, core_ids=list(range(NCORES)))
    return assemble([res.results[c]["out"] for c in range(NCORES)])


# revision 13
# speedup vs baseline: 1.5205x; 1.2680x over previous
"""GAT layer kernel for Trainium2 (8 NeuronCores, SPMD, no collectives).

Math (reference):
    att = h @ h.T / sqrt(256)
    A = softmax(where(adj>0, att, -9e15), axis=1)
    A = (A + I) * 0.5; rows < k (k = nnz(adj[:,0])) overwritten with I
    out = relu(A @ (h @ W.T + b))

Algorithm (v2 — h-space flash attention, fp8 DoubleRow, diag extracted):
  Since softmax rows sum to 1 exactly, A @ (h W^T + b) = (A @ h) W^T + b,
  so the O(N^2) matmuls run in h-space and W is applied once at the end:
    rows [0,k):  out = relu(h @ W^T + b)
    rows [k,N):  out = relu((0.5*num/S + 0.5*h_i) @ W^T + b)
        num = num_off + r_i*h_i,  S = S_off + r_i
        num_off = sum_{j!=i} em[j,i]*h_j,   S_off = sum_{j!=i} em[j,i]
        em = exp(att/16 - 1.5) * mask_offdiag   (bias -1.5 recenters into
        fp8e4 range; it cancels in num/S)
        r_i = adj[i,i] * exp(|h_i|^2/16 - 1.5)  (host, f32 exact — the
        diagonal is the only entry that can overflow fp8, so it is zeroed
        in the mask and re-added exactly)
  Both big matmuls run in fp8e4 with perf_mode=DoubleRow (K=256 packed as
  [128,2,*]); exp is batched per jc-pair [128,1024] on ScalarE with fp8
  output; the mask multiply is one fp8 tensor_tensor per pair.
  Inputs stream in consumption order so compute starts ~1us in.

Sharding: identity rows and attention rows split evenly across 8 cores;
every core runs the same NEFF on different input slices.
"""

import math
import os
import sys

for _p in ("/opt/trn_rl_repo", "/root/.axon_site/_ro/trn_rl_repo"):
    if os.path.isdir(_p) and _p not in sys.path:
        sys.path.append(_p)

import numpy as np
import orjson

import concourse.bass as bass
import concourse.tile as tile
from concourse import mybir

F32 = mybir.dt.float32
F16 = mybir.dt.float16
BF16 = mybir.dt.bfloat16
F8 = mybir.dt.float8e4
F8E5 = mybir.dt.float8e5
I8 = mybir.dt.int8
I32 = mybir.dt.int32
DRMODE = mybir.MatmulPerfMode.DoubleRow

N = 8192
D = 256
NCORES = 8
NJC = N // 128  # 64 j-chunks of 128 rows
NJP = NJC // 2  # 32 j-pairs (DoubleRow K=256)
SCALE = 1.0 / 16.0
EBIAS = -1.5  # exp recentering; cancels in num/S


def _spill_waits(nc, max_sync=2):
    """Walrus rejects instructions with more sync commands than the lowered
    ISA struct can hold (2 for compute/DMA, 1 for NoOp/Drain). Tile can emit
    more. Move excess waits onto injected NoOps preceding the instruction
    (same engine, executes in order, so semantics are preserved)."""
    bir = orjson.loads(nc.to_json_bytes())
    for fn in bir["functions"]:
        for bb in fn["blocks"]:
            insts = bb.get("instructions") or []
            out = []
            for inst in insts:
                si = inst.get("sync_info")
                if si:
                    waits = si.get("on_wait") or []
                    upds = si.get("on_update") or []
                    lim = 1 if inst["opcode"] in ("NoOp", "Drain") else max_sync
                    cap = max(0, lim - len(upds))
                    if len(waits) > cap:
                        extra = waits[cap:]
                        si["on_wait"] = waits[:cap]
                        for ci, w in enumerate(extra):
                            out.append(
                                {
                                    "engine": inst["engine"],
                                    "ins": [],
                                    "outs": [],
                                    "name": f"{inst['name']}-sw{ci}",
                                    "opcode": "NoOp",
                                    "sync_info": {"on_wait": [w], "on_update": []},
                                    "debug": inst.get("debug", 0),
                                }
                            )
                out.append(inst)
            bb["instructions"] = out
    blob = orjson.dumps(bir)
    nc.to_json_bytes = lambda: blob


def _build(nid, nis, depth=2):
    """Build the SPMD program. nid/nis = number of 128-row identity /
    attention sub-tiles per core."""
    kid = nid * 128
    rpad = nis * 128
    own = kid + rpad
    ngroups = math.ceil(rpad / 512) if nis else 0

    nc = bass.Bass("TRN2", target_bir_lowering=False, debug=False, num_devices=NCORES)

    # --- dram inputs ---
    # idwt: per-partition [dc, kid | 256] fp16 = hIdT and W^T packed
    idwt_d = nc.dram_tensor(
        "idwt", [128, 2 * (kid + 256)], F16, kind="ExternalInput"
    ).ap()
    bbrb_d = nc.dram_tensor(
        "bbrb", [128, 256 + (rpad if nis else 0)], F32, kind="ExternalInput"
    ).ap()
    if nis:
        # fp8 DoubleRow operands, d-pair = (p, p+128)
        hTo8_d = nc.dram_tensor("hTo8", [128, 2 * rpad], F8, kind="ExternalInput").ap()
        hT8_d = nc.dram_tensor("hT8", [128, 2 * N], F8, kind="ExternalInput").ap()
        # mask (diag zeroed) [p, jc, i] and MM2 weights h rows [p, jc, d]
        m8_d = nc.dram_tensor("m8", [128, NJC * rpad], I8, kind="ExternalInput").ap()
        hpd_d = nc.dram_tensor("hpd", [128, NJC * 256], F8, kind="ExternalInput").ap()
        # end-phase: H = 0.5*h_att^T fp16 packed [128, 2, rpad]
        H_d = nc.dram_tensor("H", [128, 2 * rpad], F16, kind="ExternalInput").ap()
    out_d = nc.dram_tensor("out", [own, 256], F32, kind="ExternalOutput").ap()

    with tile.TileContext(nc) as tc:
        with (
            tc.tile_pool(name="big", bufs=1) as big,
            tc.tile_pool(name="work", bufs=3) as work,
            tc.tile_pool(name="fin", bufs=2) as fin,
            tc.tile_pool(name="att_ps", bufs=2, space="PSUM") as attp,
            tc.tile_pool(name="acc_ps", bufs=1, space="PSUM") as accp,
        ):
            # --- loads, in consumption order ---
            ebias_t = big.tile([128, 1], F32, tag="ebias")
            nc.vector.memset(ebias_t[:], EBIAS)
            if nis:
                hTo8_t = big.tile([128, 2, rpad], F8, tag="hTo8")
                nc.sync.dma_start(
                    hTo8_t[:].rearrange("p v n -> p (v n)"), hTo8_d[:, :]
                )
                hT8_r = hT8_d.rearrange("p (v n) -> p v n", v=2)
                m8_r = m8_d.rearrange("p (a n) -> p a n", a=NJC)
                hpd_r = hpd_d.rearrange("p (a n) -> p a n", a=NJC)
                hT8_t = [None] * 4  # 16 jc per chunk
                m8_t = [None] * 4  # 16 jc per chunk
                hpd_t = [None] * 2  # 32 jc per chunk
                def _ld_hT8(c):
                    t = big.tile([128, 2, 2048], F8, tag=f"hT8_{c}", name=f"hT8_{c}")
                    nc.sync.dma_start(t[:], hT8_r[:, :, c * 2048 : (c + 1) * 2048])
                    hT8_t[c] = t
                def _ld_m8(c):
                    t = big.tile([128, 16, rpad], I8, tag=f"m8_{c}", name=f"m8_{c}")
                    nc.sync.dma_start(t[:], m8_r[:, c * 16 : (c + 1) * 16, :])
                    m8_t[c] = t
                def _ld_hpd(c):
                    t = big.tile([128, 32, 256], F8, tag=f"hpd_{c}", name=f"hpd_{c}")
                    nc.sync.dma_start(t[:], hpd_r[:, c * 32 : (c + 1) * 32, :])
                    hpd_t[c] = t
                _ld_hT8(0)
                _ld_hpd(0)
                _ld_m8(0)
            idwt_t = big.tile([128, 2, kid + 256], F16, tag="idwt")
            nc.sync.dma_start(idwt_t[:].rearrange("p v n -> p (v n)"), idwt_d[:, :])
            hIdT_t = [idwt_t[:, dc, 0:kid] for dc in range(2)]
            WT16_t = [idwt_t[:, dc, kid : kid + 256] for dc in range(2)]
            bbrb_t = big.tile([128, 256 + (rpad if nis else 0)], F32, tag="bbrb")
            nc.sync.dma_start(bbrb_t[:], bbrb_d[:, :])
            bb_t = bbrb_t[:, 0:256]
            if nis:
                rb_t = bbrb_t[:, 256:]
                H_t2 = big.tile([128, 2, rpad], F16, tag="H")
                nc.sync.dma_start(H_t2[:].rearrange("p v n -> p (v n)"), H_d[:, :])
                H_t = [H_t2[:, dc, :] for dc in range(2)]
                _ld_hT8(1)
                _ld_m8(1)
                _ld_hpd(1)
                _ld_hT8(2)
                _ld_m8(2)
                _ld_hT8(3)
                _ld_m8(3)
                ones8 = big.tile([128, 2, 16], F8, tag="ones8")
                nc.vector.memset(ones8[:], 1.0)
                ones32 = big.tile([1, 128], F32, tag="ones32")
                nc.vector.memset(ones32[:], 1.0)
                # PE warmup - DMA-independent dummy matmuls to engage HAM
                wz = big.tile([128, 2, 512], F8, tag="wz")
                nc.vector.memset(wz[:].rearrange("p v n -> p (v n)"), 0.0)
                warm_ps = accp.tile([128, 512], F32, tag="warm")
                for _w in range(10):
                    nc.tensor.matmul(
                        warm_ps[0:16, :], ones8[:, :, 0:16], wz[:],
                        start=True, stop=True, perf_mode=DRMODE,
                        skip_group_check=True,
                    )

            # --- id phase: out rows [0,kid) = relu(h @ W^T + b) ---
            for it in range(nid):
                ps_t = accp.tile([128, 512], F32, tag=f"acc{it % 2}")
                ps = ps_t[:, 0:256]
                for dc in range(2):
                    nc.tensor.matmul(
                        ps,
                        hIdT_t[dc][:, it * 128 : (it + 1) * 128],
                        WT16_t[dc],
                        start=(dc == 0),
                        stop=(dc == 1),
                    )
                tmp = fin.tile([128, 256], F32, tag="id_tmp", bufs=4)
                nc.vector.tensor_tensor(tmp[:], ps, bb_t, op=mybir.AluOpType.add)
                o_t = fin.tile([128, 256], F32, tag="id_o", bufs=max(nid, 1))
                nc.vector.tensor_scalar_max(o_t[:], tmp[:], 0.0)
                nc.sync.dma_start(out_d[it * 128 : (it + 1) * 128, :], o_t[:])

            # --- hoisted loop-invariant: t0 = rb * H (diag term) ---
            t0_t = []
            if nis:
                for dc in range(2):
                    t0 = big.tile([128, rpad], F32, tag=f"t0_{dc}", name=f"t0_{dc}")
                    nc.vector.tensor_tensor(
                        t0[:], rb_t, H_t[dc], op=mybir.AluOpType.mult
                    )
                    t0_t.append(t0)

            # --- attention phase ---
            for g in range(ngroups):
                i_lo = g * 512
                iw = min(512, rpad - i_lo)
                acc_d = [
                    accp.tile([128, 512], F32, tag=f"acc{dc}", name=f"acc{dc}")
                    for dc in range(2)
                ]
                s_ps = accp.tile([16, 512], F32, tag="s_ps")
                pend = []

                def emit_mm2(t, em_t, g=g, iw=iw, acc_d=acc_d, s_ps=s_ps):
                    for dc in range(2):
                        nc.tensor.matmul(
                            acc_d[dc][:, 0:iw],
                            hpd_t[t // 16][:, (t % 16) * 2 : (t % 16) * 2 + 2,
                                           dc * 128 : (dc + 1) * 128],
                            em_t[:, :, 0:iw],
                            start=(t == 0),
                            stop=(t == NJP - 1),
                            perf_mode=DRMODE,
                        )
                    nc.tensor.matmul(
                        s_ps[:, 0:iw],
                        ones8[:, :, 0:16],
                        em_t[:, :, 0:iw],
                        start=(t == 0),
                        stop=(t == NJP - 1),
                        perf_mode=DRMODE,
                    )

                for t in range(NJP):
                    att_ps = attp.tile([128, 1024], F32, tag="att")
                    for v in range(2):
                        jc = 2 * t + v
                        nc.tensor.matmul(
                            att_ps[:, v * 512 : v * 512 + iw],
                            hT8_t[jc // 16][:, :, (jc % 16) * 128 : (jc % 16 + 1) * 128],
                            hTo8_t[:, :, i_lo : i_lo + iw],
                            start=True,
                            stop=True,
                            perf_mode=DRMODE,
                        )
                    e8_t = work.tile([128, 1024], F8E5, tag="e8")
                    nc.scalar.activation(
                        e8_t[:],
                        att_ps[:],
                        mybir.ActivationFunctionType.Exp,
                        scale=SCALE,
                        bias=ebias_t[:],
                    )
                    em_t = work.tile([128, 2, 512], F8E5, tag="em")
                    nc.vector.tensor_tensor(
                        em_t[:, :, :].bitcast(I32),
                        e8_t[:].rearrange("p (v n) -> p v n", v=2).bitcast(I32),
                        m8_t[t // 8][:, (t % 8) * 2 : (t % 8) * 2 + 2,
                                     i_lo : i_lo + 512].bitcast(I32),
                        op=mybir.AluOpType.bitwise_and,
                    )
                    pend.append((t, em_t))
                    if len(pend) > depth:
                        emit_mm2(*pend.pop(0))
                for item in pend:
                    emit_mm2(*item)

                # --- end phase for this group ---
                # S_full = S_off + r ; recipS broadcast via f32 ones matmul
                s_full = fin.tile([16, 512], F32, tag="s_full")
                nc.vector.tensor_tensor(
                    s_full[:, 0:iw], s_ps[:, 0:iw], bbrb_t[0:16, 256 + i_lo : 256 + i_lo + iw],
                    op=mybir.AluOpType.add,
                )
                s_rec = fin.tile([16, 512], F32, tag="s_rec")
                nc.vector.reciprocal(s_rec[:, 0:iw], s_full[:, 0:iw])
                rec_bc = attp.tile([128, 1024], F32, tag="att")
                nc.tensor.matmul(
                    rec_bc[:, 0:iw], ones32[:], s_rec[0:1, 0:iw], start=True,
                    stop=True,
                )
                blend = []
                for dc in range(2):
                    t1 = fin.tile([128, 512], F32, tag="bl_t1")
                    nc.vector.scalar_tensor_tensor(
                        t1[:, 0:iw],
                        acc_d[dc][:, 0:iw],
                        0.5,
                        t0_t[dc][:, i_lo : i_lo + iw],
                        op0=mybir.AluOpType.mult,
                        op1=mybir.AluOpType.add,
                    )
                    t2 = fin.tile([128, 512], F32, tag="bl_t2")
                    nc.vector.tensor_tensor(
                        t2[:, 0:iw], t1[:, 0:iw], rec_bc[:, 0:iw],
                        op=mybir.AluOpType.mult,
                    )
                    bl = fin.tile([128, 512], F16, tag=f"blend{dc}", name=f"bl{dc}")
                    nc.vector.tensor_tensor(
                        bl[:, 0:iw], t2[:, 0:iw], H_t2[:, dc, i_lo : i_lo + iw],
                        op=mybir.AluOpType.add,
                    )
                    blend.append(bl)
                # W apply + bias + relu + store
                for it in range(iw // 128):
                    ps_t = accp.tile([128, 512], F32, tag=f"acc{it % 2}")
                    ps = ps_t[:, 0:256]
                    for dc in range(2):
                        nc.tensor.matmul(
                            ps,
                            blend[dc][:, it * 128 : (it + 1) * 128],
                            WT16_t[dc],
                            start=(dc == 0),
                            stop=(dc == 1),
                        )
                    tmp = fin.tile([128, 256], F32, tag="w_tmp")
                    nc.vector.tensor_tensor(
                        tmp[:], ps, bb_t, op=mybir.AluOpType.add
                    )
                    o_t = fin.tile([128, 256], F32, tag="w_o")
                    nc.vector.tensor_scalar_max(o_t[:], tmp[:], 0.0)
                    r0 = kid + i_lo + it * 128
                    nc.sync.dma_start(out_d[r0 : r0 + 128, :], o_t[:])

    _spill_waits(nc)
    return nc


_CACHE = {}


def _prepare(h, adj, W, b):
    """Host-side sharding + layout prep. Returns (nc, in_maps, assemble)."""
    h = np.asarray(h, dtype=np.float32)
    adj = np.asarray(adj)
    W = np.asarray(W, dtype=np.float32)
    b = np.asarray(b, dtype=np.float32)

    k = int(np.count_nonzero(adj[:, 0]))
    nid = (k + NCORES * 128 - 1) // (NCORES * 128)
    nis = (N - k + NCORES * 128 - 1) // (NCORES * 128)
    key = (nid, nis)
    if key not in _CACHE:
        _CACHE[key] = _build(nid, nis)
    nc = _CACHE[key]

    kid = nid * 128
    rpad = nis * 128

    f8 = mybir.dt.np(F8)
    adj8 = (adj != 0)
    # shared across cores
    h8 = h.astype(f8)  # [N, 256] fp8
    hT8 = np.ascontiguousarray(h8.T)  # [256, N]
    hT8_dr = hT8.reshape(2, 128, N).transpose(1, 0, 2).reshape(128, 2 * N)
    hT8_dr = np.ascontiguousarray(hT8_dr)
    # hpd[p, jc, d] = h8[jc*128 + p, d]
    hpd = np.ascontiguousarray(
        h8.reshape(NJC, 128, 256).transpose(1, 0, 2)
    ).reshape(128, NJC * 256)
    WT16 = np.ascontiguousarray(W.T).astype(np.float16)
    bb = np.broadcast_to(b, (128, 256)).astype(np.float32).copy()
    # diagonal r_i = adj_ii * exp(|h_i|^2 * SCALE + EBIAS)  (f32 exact)
    dot_ii = np.einsum("nd,nd->n", h, h)
    r_full = np.where(adj8.diagonal(), np.exp(dot_ii * SCALE + EBIAS), 0.0).astype(
        np.float32
    )

    hT32 = h.T  # [256, N] f32

    in_maps = []
    row_lists = []
    for c in range(NCORES):
        id_rows = np.arange(c * kid, (c + 1) * kid)
        id_valid = id_rows < k
        id_rows = np.where(id_valid, id_rows, 0)
        att_rows = np.arange(k + c * rpad, k + (c + 1) * rpad)
        att_valid = att_rows < N
        att_rows_c = np.where(att_valid, att_rows, 0)
        row_lists.append((id_rows, id_valid, att_rows_c, att_valid))

        kid_c = kid
        hIdT_c = hT32[:, id_rows].astype(np.float16)  # [256, kid]
        idwt = np.empty((128, 2, kid_c + 256), dtype=np.float16)
        for dc in range(2):
            idwt[:, dc, :kid_c] = hIdT_c[dc * 128 : (dc + 1) * 128]
            idwt[:, dc, kid_c:] = WT16[dc * 128 : (dc + 1) * 128]
        im = {"idwt": idwt.reshape(128, -1)}
        if nis:
            hTo8 = hT8[:, att_rows_c]  # [256, rpad]
            im["hTo8"] = np.ascontiguousarray(
                hTo8.reshape(2, 128, rpad).transpose(1, 0, 2)
            ).reshape(128, 2 * rpad)
            im["hT8"] = hT8_dr
            im["hpd"] = hpd
            # mask [p, jc, i] = adj[att_row_i, jc*128+p], diag zeroed
            mT = adj8[att_rows_c, :].T.astype(np.int8)  # [N, rpad]
            nval = int(att_valid.sum())
            if nval < rpad:
                mT[:, nval:] = 0
            mT[att_rows_c[:nval], np.arange(nval)] = 0  # zero diagonal
            m8 = (
                np.ascontiguousarray(
                    mT.reshape(NJC, 128, rpad).transpose(1, 0, 2)
                )
                * np.int8(-1)
            ).reshape(128, NJC * rpad)
            im["m8"] = m8
            Hm = (0.5 * hT32[:, att_rows_c]).astype(np.float16)  # [256, rpad]
            im["H"] = np.ascontiguousarray(
                Hm.reshape(2, 128, rpad).transpose(1, 0, 2)
            ).reshape(128, 2 * rpad)
            r_c = np.where(att_valid, r_full[att_rows_c], 1.0).astype(np.float32)
            bbrb = np.empty((128, 256 + rpad), dtype=np.float32)
            bbrb[:, :256] = bb
            bbrb[:, 256:] = r_c[None, :]
            im["bbrb"] = bbrb
        else:
            im["bbrb"] = bb
        in_maps.append(im)

    def assemble(outs):
        out = np.empty((N, 256), dtype=np.float32)
        for c in range(NCORES):
            id_rows, id_valid, att_rows_c, att_valid = row_lists[c]
            o = outs[c]
            if id_valid.any():
                out[id_rows[id_valid]] = o[:kid][id_valid]
            if att_valid.any():
                out[att_rows_c[att_valid]] = o[kid:][att_valid]
        return out

    return nc, in_maps, assemble


def kernel(h, adj, W, b):
    nc, in_maps, assemble = _prepare(h, adj, W, b)

    from concourse.bass_utils import run_bass_kernel_spmd

    res = run_bass_kernel_spmd(nc, in_maps, core_ids=list(range(NCORES)))
    return assemble([res.results[c]["out"] for c in range(NCORES)])


# revision 17
# speedup vs baseline: 1.5996x; 1.0520x over previous
"""GAT layer kernel for Trainium2 (8 NeuronCores, SPMD, no collectives).

Math (reference):
    att = h @ h.T / sqrt(256)
    A = softmax(where(adj>0, att, -9e15), axis=1)
    A = (A + I) * 0.5; rows < k (k = nnz(adj[:,0])) overwritten with I
    out = relu(A @ (h @ W.T + b))

Algorithm (v2 — h-space flash attention, fp8 DoubleRow, diag extracted):
  Since softmax rows sum to 1 exactly, A @ (h W^T + b) = (A @ h) W^T + b,
  so the O(N^2) matmuls run in h-space and W is applied once at the end:
    rows [0,k):  out = relu(h @ W^T + b)
    rows [k,N):  out = relu((0.5*num/S + 0.5*h_i) @ W^T + b)
        num = num_off + r_i*h_i,  S = S_off + r_i
        num_off = sum_{j!=i} em[j,i]*h_j,   S_off = sum_{j!=i} em[j,i]
        em = exp(att/16 - 1.5) * mask_offdiag   (bias -1.5 recenters into
        fp8e4 range; it cancels in num/S)
        r_i = adj[i,i] * exp(|h_i|^2/16 - 1.5)  (host, f32 exact — the
        diagonal is the only entry that can overflow fp8, so it is zeroed
        in the mask and re-added exactly)
  Both big matmuls run in fp8e4 with perf_mode=DoubleRow (K=256 packed as
  [128,2,*]); exp is batched per jc-pair [128,1024] on ScalarE with fp8
  output; the mask multiply is one fp8 tensor_tensor per pair.
  Inputs stream in consumption order so compute starts ~1us in.

Sharding: identity rows and attention rows split evenly across 8 cores;
every core runs the same NEFF on different input slices.
"""

import math
import os
import sys

for _p in ("/opt/trn_rl_repo", "/root/.axon_site/_ro/trn_rl_repo"):
    if os.path.isdir(_p) and _p not in sys.path:
        sys.path.append(_p)

import numpy as np
import orjson

import concourse.bass as bass
import concourse.tile as tile
from concourse import mybir

F32 = mybir.dt.float32
F16 = mybir.dt.float16
BF16 = mybir.dt.bfloat16
F8 = mybir.dt.float8e4
F8E5 = mybir.dt.float8e5
I8 = mybir.dt.int8
I32 = mybir.dt.int32
DRMODE = mybir.MatmulPerfMode.DoubleRow

N = 8192
D = 256
NCORES = 8
NJC = N // 128  # 64 j-chunks of 128 rows
NJP = NJC // 2  # 32 j-pairs (DoubleRow K=256)
SCALE = 1.0 / 16.0
EBIAS = -1.5  # exp recentering; cancels in num/S


def _spill_waits(nc, max_sync=2):
    """Walrus rejects instructions with more sync commands than the lowered
    ISA struct can hold (2 for compute/DMA, 1 for NoOp/Drain). Tile can emit
    more. Move excess waits onto injected NoOps preceding the instruction
    (same engine, executes in order, so semantics are preserved)."""
    bir = orjson.loads(nc.to_json_bytes())
    for fn in bir["functions"]:
        for bb in fn["blocks"]:
            insts = bb.get("instructions") or []
            out = []
            for inst in insts:
                si = inst.get("sync_info")
                if si:
                    waits = si.get("on_wait") or []
                    upds = si.get("on_update") or []
                    lim = 1 if inst["opcode"] in ("NoOp", "Drain") else max_sync
                    cap = max(0, lim - len(upds))
                    if len(waits) > cap:
                        extra = waits[cap:]
                        si["on_wait"] = waits[:cap]
                        for ci, w in enumerate(extra):
                            out.append(
                                {
                                    "engine": inst["engine"],
                                    "ins": [],
                                    "outs": [],
                                    "name": f"{inst['name']}-sw{ci}",
                                    "opcode": "NoOp",
                                    "sync_info": {"on_wait": [w], "on_update": []},
                                    "debug": inst.get("debug", 0),
                                }
                            )
                out.append(inst)
            bb["instructions"] = out
    blob = orjson.dumps(bir)
    nc.to_json_bytes = lambda: blob


def _build(nid, nis, depth=2):
    """Build the SPMD program. nid/nis = number of 128-row identity /
    attention sub-tiles per core."""
    kid = nid * 128
    rpad = nis * 128
    own = kid + rpad
    ngroups = math.ceil(rpad / 512) if nis else 0

    nc = bass.Bass("TRN2", target_bir_lowering=False, debug=False, num_devices=NCORES)

    # --- dram inputs ---
    # idwt: per-partition [dc, kid | 256] fp16 = hIdT and W^T packed
    idwt_d = nc.dram_tensor(
        "idwt", [128, 2 * (kid + 256)], F16, kind="ExternalInput"
    ).ap()
    bbrb_d = nc.dram_tensor(
        "bbrb", [128, 256 + (rpad if nis else 0)], F32, kind="ExternalInput"
    ).ap()
    if nis:
        # fp8 DoubleRow operands, d-pair = (p, p+128)
        hTo8_d = nc.dram_tensor("hTo8", [128, 2 * rpad], F8, kind="ExternalInput").ap()
        hT8_d = nc.dram_tensor("hT8", [128, 2 * N], F8, kind="ExternalInput").ap()
        # mask (diag zeroed) [p, jc, i] and MM2 weights h rows [p, jc, d]
        m8_d = nc.dram_tensor("m8", [128, NJC * rpad], I8, kind="ExternalInput").ap()
        hpd_d = nc.dram_tensor("hpd", [128, NJC * 256], F8, kind="ExternalInput").ap()
        # end-phase: H = 0.5*h_att^T fp16 packed [128, 2, rpad]
        H_d = nc.dram_tensor("H", [128, 2 * rpad], F16, kind="ExternalInput").ap()
    out_d = nc.dram_tensor("out", [own, 256], F32, kind="ExternalOutput").ap()

    with tile.TileContext(nc) as tc:
        with (
            tc.tile_pool(name="big", bufs=1) as big,
            tc.tile_pool(name="work", bufs=3) as work,
            tc.tile_pool(name="fin", bufs=2) as fin,
            tc.tile_pool(name="att_ps", bufs=2, space="PSUM") as attp,
            tc.tile_pool(name="acc_ps", bufs=1, space="PSUM") as accp,
        ):
            # --- loads, in consumption order ---
            ebias_t = big.tile([128, 1], F32, tag="ebias")
            nc.vector.memset(ebias_t[:], EBIAS)
            if nis:
                hTo8_t = big.tile([128, 2, rpad], F8, tag="hTo8")
                nc.sync.dma_start(
                    hTo8_t[:].rearrange("p v n -> p (v n)"), hTo8_d[:, :]
                )
                hT8_r = hT8_d.rearrange("p (v n) -> p v n", v=2)
                m8_r = m8_d.rearrange("p (a n) -> p a n", a=NJC)
                hpd_r = hpd_d.rearrange("p (a n) -> p a n", a=NJC)
                hT8_t = [None] * 4  # 16 jc per chunk
                m8_t = [None] * 4  # 16 jc per chunk
                hpd_t = [None] * 2  # 32 jc per chunk
                def _ld_hT8(c):
                    t = big.tile([128, 2, 2048], F8, tag=f"hT8_{c}", name=f"hT8_{c}")
                    nc.sync.dma_start(t[:], hT8_r[:, :, c * 2048 : (c + 1) * 2048])
                    hT8_t[c] = t
                def _ld_m8(c):
                    t = big.tile([128, 16, rpad], I8, tag=f"m8_{c}", name=f"m8_{c}")
                    nc.sync.dma_start(t[:], m8_r[:, c * 16 : (c + 1) * 16, :])
                    m8_t[c] = t
                def _ld_hpd(c):
                    t = big.tile([128, 32, 256], F8, tag=f"hpd_{c}", name=f"hpd_{c}")
                    nc.sync.dma_start(t[:], hpd_r[:, c * 32 : (c + 1) * 32, :])
                    hpd_t[c] = t
                _ld_hT8(0)
                _ld_hpd(0)
                _ld_m8(0)
            bbrb_t = big.tile([128, 256 + (rpad if nis else 0)], F32, tag="bbrb")
            nc.sync.dma_start(bbrb_t[:], bbrb_d[:, :])
            bb_t = bbrb_t[:, 0:256]
            if nis:
                rb_t = bbrb_t[:, 256:]
                H_t2 = big.tile([128, 2, rpad], F16, tag="H")
                nc.sync.dma_start(H_t2[:].rearrange("p v n -> p (v n)"), H_d[:, :])
                H_t = [H_t2[:, dc, :] for dc in range(2)]
                _ld_hT8(1)
                _ld_m8(1)
                _ld_hpd(1)
                _ld_hT8(2)
                _ld_m8(2)
                _ld_hT8(3)
                _ld_m8(3)
            idwt_t = big.tile([128, 2, kid + 256], F16, tag="idwt")
            nc.sync.dma_start(idwt_t[:].rearrange("p v n -> p (v n)"), idwt_d[:, :])
            hIdT_t = [idwt_t[:, dc, 0:kid] for dc in range(2)]
            WT16_t = [idwt_t[:, dc, kid : kid + 256] for dc in range(2)]
            if nis:
                ones8 = big.tile([128, 2, 16], F8, tag="ones8")
                nc.vector.memset(ones8[:], 1.0)
                onesb = big.tile([1, 128], BF16, tag="onesb")
                nc.vector.memset(onesb[:], 1.0)
                # PE warmup - DMA-independent dummy matmuls to engage HAM
                wz = big.tile([128, 2, 512], F8, tag="wz")
                nc.vector.memset(wz[:].rearrange("p v n -> p (v n)"), 0.0)
                warm_ps = accp.tile([128, 512], F32, tag="warm")
                for _w in range(10):
                    nc.tensor.matmul(
                        warm_ps[0:16, :], ones8[:, :, 0:16], wz[:],
                        start=True, stop=True, perf_mode=DRMODE,
                        skip_group_check=True,
                    )

            # --- attention phase ---
            for g in range(ngroups):
                i_lo = g * 512
                iw = min(512, rpad - i_lo)
                acc_d = [
                    accp.tile([128, 512], F32, tag=f"acc{dc}", name=f"acc{dc}")
                    for dc in range(2)
                ]
                s_ps = accp.tile([16, 512], F32, tag="s_ps")
                pend = []

                def emit_mm2(t, em_t, g=g, iw=iw, acc_d=acc_d, s_ps=s_ps):
                    for dc in range(2):
                        nc.tensor.matmul(
                            acc_d[dc][:, 0:iw],
                            hpd_t[t // 16][:, (t % 16) * 2 : (t % 16) * 2 + 2,
                                           dc * 128 : (dc + 1) * 128],
                            em_t[:, :, 0:iw],
                            start=(t == 0),
                            stop=(t == NJP - 1),
                            perf_mode=DRMODE,
                        )
                    nc.tensor.matmul(
                        s_ps[:, 0:iw],
                        ones8[:, :, 0:16],
                        em_t[:, :, 0:iw],
                        start=(t == 0),
                        stop=(t == NJP - 1),
                        perf_mode=DRMODE,
                    )

                for t in range(NJP):
                    att_ps = attp.tile([128, 1024], F32, tag="att")
                    for v in range(2):
                        jc = 2 * t + v
                        nc.tensor.matmul(
                            att_ps[:, v * 512 : v * 512 + iw],
                            hT8_t[jc // 16][:, :, (jc % 16) * 128 : (jc % 16 + 1) * 128],
                            hTo8_t[:, :, i_lo : i_lo + iw],
                            start=True,
                            stop=True,
                            perf_mode=DRMODE,
                        )
                    e8_t = work.tile([128, 1024], F8E5, tag="e8")
                    nc.scalar.activation(
                        e8_t[:],
                        att_ps[:],
                        mybir.ActivationFunctionType.Exp,
                        scale=SCALE,
                        bias=ebias_t[:],
                    )
                    em_t = work.tile([128, 2, 512], F8E5, tag="em")
                    nc.vector.tensor_tensor(
                        em_t[:, :, :].bitcast(I32),
                        e8_t[:].rearrange("p (v n) -> p v n", v=2).bitcast(I32),
                        m8_t[t // 8][:, (t % 8) * 2 : (t % 8) * 2 + 2,
                                     i_lo : i_lo + 512].bitcast(I32),
                        op=mybir.AluOpType.bitwise_and,
                    )
                    pend.append((t, em_t))
                    if len(pend) > depth:
                        emit_mm2(*pend.pop(0))
                for item in pend:
                    emit_mm2(*item)

                # t0 = rb * H (diag term) - runs in the tail window
                t0_t = []
                for dc in range(2):
                    t0 = big.tile([128, rpad], F32, tag=f"t0_{dc}", name=f"t0_{dc}")
                    nc.vector.tensor_tensor(
                        t0[:], rb_t, H_t[dc], op=mybir.AluOpType.mult
                    )
                    t0_t.append(t0)

                # id phase: out rows [0,kid) = relu(h @ W^T + b)
                # (PE is otherwise idle while 1/S is computed)
                if g == ngroups - 1:
                    for it in range(nid):
                        id_ps_t = accp.tile([128, 512], F32, tag=f"acc{it % 2}", name="id_ps_t")
                        id_ps = id_ps_t[:, 0:256]
                        for dc in range(2):
                            nc.tensor.matmul(
                                id_ps,
                                hIdT_t[dc][:, it * 128 : (it + 1) * 128],
                                WT16_t[dc],
                                start=(dc == 0),
                                stop=(dc == 1),
                            )
                        tmp = fin.tile([128, 256], F32, tag="id_tmp", bufs=4)
                        nc.vector.tensor_tensor(
                            tmp[:], id_ps, bb_t, op=mybir.AluOpType.add
                        )
                        o_t = fin.tile([128, 256], F32, tag="id_o", bufs=max(nid, 1))
                        nc.vector.tensor_scalar_max(o_t[:], tmp[:], 0.0)
                        nc.sync.dma_start(
                            out_d[it * 128 : (it + 1) * 128, :], o_t[:]
                        )

                # --- end phase for this group ---
                # S_full = S_off + r ; recipS broadcast via f32 ones matmul
                s_full = fin.tile([16, 512], F32, tag="s_full")
                nc.vector.tensor_tensor(
                    s_full[:, 0:iw], s_ps[:, 0:iw], bbrb_t[0:16, 256 + i_lo : 256 + i_lo + iw],
                    op=mybir.AluOpType.add,
                )
                s_ln = fin.tile([16, 512], F32, tag="s_ln")
                nc.scalar.activation(
                    s_ln[:, 0:iw], s_full[:, 0:iw],
                    mybir.ActivationFunctionType.Ln,
                )
                s_rec = fin.tile([16, 512], BF16, tag="s_rec")
                nc.scalar.activation(
                    s_rec[:, 0:iw], s_ln[:, 0:iw],
                    mybir.ActivationFunctionType.Exp, scale=-1.0,
                )
                rec_bc = attp.tile([128, 1024], F32, tag="att")
                nc.tensor.matmul(
                    rec_bc[:, 0:iw], onesb[:], s_rec[0:1, 0:iw], start=True,
                    stop=True,
                )
                blend = []
                for dc in range(2):
                    t1 = fin.tile([128, 512], F32, tag="bl_t1")
                    nc.vector.scalar_tensor_tensor(
                        t1[:, 0:iw],
                        acc_d[dc][:, 0:iw],
                        0.5,
                        t0_t[dc][:, i_lo : i_lo + iw],
                        op0=mybir.AluOpType.mult,
                        op1=mybir.AluOpType.add,
                    )
                    t2 = fin.tile([128, 512], F32, tag="bl_t2")
                    nc.vector.tensor_tensor(
                        t2[:, 0:iw], t1[:, 0:iw], rec_bc[:, 0:iw],
                        op=mybir.AluOpType.mult,
                    )
                    bl = fin.tile([128, 512], F16, tag=f"blend{dc}", name=f"bl{dc}")
                    nc.vector.tensor_tensor(
                        bl[:, 0:iw], t2[:, 0:iw], H_t2[:, dc, i_lo : i_lo + iw],
                        op=mybir.AluOpType.add,
                    )
                    blend.append(bl)
                # W apply + bias + relu + store
                for it in range(iw // 128):
                    ps_t = accp.tile([128, 512], F32, tag=f"acc{it % 2}")
                    ps = ps_t[:, 0:256]
                    for dc in range(2):
                        nc.tensor.matmul(
                            ps,
                            blend[dc][:, it * 128 : (it + 1) * 128],
                            WT16_t[dc],
                            start=(dc == 0),
                            stop=(dc == 1),
                        )
                    tmp = fin.tile([128, 256], F32, tag="w_tmp")
                    nc.vector.tensor_tensor(
                        tmp[:], ps, bb_t, op=mybir.AluOpType.add
                    )
                    o_t = fin.tile([128, 256], F32, tag="w_o")
                    nc.vector.tensor_scalar_max(o_t[:], tmp[:], 0.0)
                    r0 = kid + i_lo + it * 128
                    nc.sync.dma_start(out_d[r0 : r0 + 128, :], o_t[:])

    _spill_waits(nc)
    return nc


_CACHE = {}


def _prepare(h, adj, W, b):
    """Host-side sharding + layout prep. Returns (nc, in_maps, assemble)."""
    h = np.asarray(h, dtype=np.float32)
    adj = np.asarray(adj)
    W = np.asarray(W, dtype=np.float32)
    b = np.asarray(b, dtype=np.float32)

    k = int(np.count_nonzero(adj[:, 0]))
    nid = (k + NCORES * 128 - 1) // (NCORES * 128)
    nis = (N - k + NCORES * 128 - 1) // (NCORES * 128)
    key = (nid, nis)
    if key not in _CACHE:
        _CACHE[key] = _build(nid, nis)
    nc = _CACHE[key]

    kid = nid * 128
    rpad = nis * 128

    f8 = mybir.dt.np(F8)
    adj8 = (adj != 0)
    # shared across cores
    h8 = h.astype(f8)  # [N, 256] fp8
    hT8 = np.ascontiguousarray(h8.T)  # [256, N]
    hT8_dr = hT8.reshape(2, 128, N).transpose(1, 0, 2).reshape(128, 2 * N)
    hT8_dr = np.ascontiguousarray(hT8_dr)
    # hpd[p, jc, d] = h8[jc*128 + p, d]
    hpd = np.ascontiguousarray(
        h8.reshape(NJC, 128, 256).transpose(1, 0, 2)
    ).reshape(128, NJC * 256)
    WT16 = np.ascontiguousarray(W.T).astype(np.float16)
    bb = np.broadcast_to(b, (128, 256)).astype(np.float32).copy()
    # diagonal r_i = adj_ii * exp(|h_i|^2 * SCALE + EBIAS)  (f32 exact)
    dot_ii = np.einsum("nd,nd->n", h, h)
    r_full = np.where(adj8.diagonal(), np.exp(dot_ii * SCALE + EBIAS), 0.0).astype(
        np.float32
    )

    hT32 = h.T  # [256, N] f32

    in_maps = []
    row_lists = []
    for c in range(NCORES):
        id_rows = np.arange(c * kid, (c + 1) * kid)
        id_valid = id_rows < k
        id_rows = np.where(id_valid, id_rows, 0)
        att_rows = np.arange(k + c * rpad, k + (c + 1) * rpad)
        att_valid = att_rows < N
        att_rows_c = np.where(att_valid, att_rows, 0)
        row_lists.append((id_rows, id_valid, att_rows_c, att_valid))

        kid_c = kid
        hIdT_c = hT32[:, id_rows].astype(np.float16)  # [256, kid]
        idwt = np.empty((128, 2, kid_c + 256), dtype=np.float16)
        for dc in range(2):
            idwt[:, dc, :kid_c] = hIdT_c[dc * 128 : (dc + 1) * 128]
            idwt[:, dc, kid_c:] = WT16[dc * 128 : (dc + 1) * 128]
        im = {"idwt": idwt.reshape(128, -1)}
        if nis:
            hTo8 = hT8[:, att_rows_c]  # [256, rpad]
            im["hTo8"] = np.ascontiguousarray(
                hTo8.reshape(2, 128, rpad).transpose(1, 0, 2)
            ).reshape(128, 2 * rpad)
            im["hT8"] = hT8_dr
            im["hpd"] = hpd
            # mask [p, jc, i] = adj[att_row_i, jc*128+p], diag zeroed
            mT = adj8[att_rows_c, :].T.astype(np.int8)  # [N, rpad]
            nval = int(att_valid.sum())
            if nval < rpad:
                mT[:, nval:] = 0
            mT[att_rows_c[:nval], np.arange(nval)] = 0  # zero diagonal
            m8 = (
                np.ascontiguousarray(
                    mT.reshape(NJC, 128, rpad).transpose(1, 0, 2)
                )
                * np.int8(-1)
            ).reshape(128, NJC * rpad)
            im["m8"] = m8
            Hm = (0.5 * hT32[:, att_rows_c]).astype(np.float16)  # [256, rpad]
            im["H"] = np.ascontiguousarray(
                Hm.reshape(2, 128, rpad).transpose(1, 0, 2)
            ).reshape(128, 2 * rpad)
            r_c = np.where(att_valid, r_full[att_rows_c], 1.0).astype(np.float32)
            bbrb = np.empty((128, 256 + rpad), dtype=np.float32)
            bbrb[:, :256] = bb
            bbrb[:, 256:] = r_c[None, :]
            im["bbrb"] = bbrb
        else:
            im["bbrb"] = bb
        in_maps.append(im)

    def assemble(outs):
        out = np.empty((N, 256), dtype=np.float32)
        for c in range(NCORES):
            id_rows, id_valid, att_rows_c, att_valid = row_lists[c]
            o = outs[c]
            if id_valid.any():
                out[id_rows[id_valid]] = o[:kid][id_valid]
            if att_valid.any():
                out[att_rows_c[att_valid]] = o[kid:][att_valid]
        return out

    return nc, in_maps, assemble


def kernel(h, adj, W, b):
    nc, in_maps, assemble = _prepare(h, adj, W, b)

    from concourse.bass_utils import run_bass_kernel_spmd

    res = run_bass_kernel_spmd(nc, in_maps, core_ids=list(range(NCORES)))
    return assemble([res.results[c]["out"] for c in range(NCORES)])
